# revision 1
# baseline (speedup 1.0000x reference)
"""Trainium2 Bass kernel for softmax RGB blend (pytorch3d NoLightShader).

Full inputs (N=8, H=512, W=512, K=8) are sharded batch-wise across 8
NeuronCores (one batch image per core); the blend is purely per-pixel so no
cross-core communication is needed.

Math per pixel (K faces):
    mask_k  = pix_to_face_k >= 0
    prob_k  = sigmoid(-dists_k / SIGMA) * mask_k
    alpha   = 1 - prod_k(1 - prob_k)        (via exp(sum ln(1 - prob_k)))
    z_k     = (ZFAR - zbuf_k) / (ZFAR - ZNEAR) * mask_k
    zmax    = max_k z_k                     (EPS clamp dropped: only matters
                                             for all-masked pixels, where the
                                             result is unchanged)
    w_k     = prob_k * exp((z_k - zmax) / GAMMA)
    delta   = exp((EPS - zmax) / GAMMA)
    denom   = sum_k w_k + delta
    rgb_c   = (sum_k w_k * color_kc + delta) / denom    (background = 1,1,1)
    out     = [rgb, alpha]

Raw-bass pipeline (Tile's multi-wait instructions don't compile on this
walrus, so waits are explicit single-sem instructions):
    SP  (sync) : HWDGE DMAs in/out, double-buffered input slots
    ACT (scalar): sigmoid, z-linearize, ln(1-prob), exp(zd/g), delta, prod(q)
    DVE (vector): mask, mask applies, the four K-reductions, w, w*c,
                  denom, reciprocal, rgb/alpha finalize
Per-tile op indices give deterministic semaphore thresholds; see marks below.
"""

import sys
from contextlib import ExitStack

import numpy as np

if "/opt/trn_rl_repo" not in sys.path:
    sys.path.insert(0, "/opt/trn_rl_repo")

SIGMA = 1e-4
GAMMA = 1e-4
ZNEAR = 1.0
ZFAR = 100.0
EPS = 1e-10

P = 128
K = 8
N_CORES = 8

# per-tile op counts (sem increments per tile per engine)
N_ACT = 9   # sig, zlin, lnq, ex, delta, prodq, lnd, rcp, alpha
N_DVE = 10  # prob, zinv, zmax, qsum, wsum, denom, wc, csum, t3, rgb
N_GP = 2    # zd, w


def build_program(rows, T):
    import concourse.bass as bass
    from concourse import mybir

    dt = mybir.dt
    f32 = dt.float32
    Alu = mybir.AluOpType
    Act = mybir.ActivationFunctionType
    Ax = mybir.AxisListType

    assert rows % T == 0
    n = rows // T
    TK = T * K

    nc = bass.Bass()

    zb_d = nc.dram_tensor("zbuf", [P, rows * K], f32, kind="ExternalInput")
    ds_d = nc.dram_tensor("dists", [P, rows * K], f32, kind="ExternalInput")
    pf_d = nc.dram_tensor("pix_to_face", [P, rows * K], dt.int32, kind="ExternalInput")
    pc_d = nc.dram_tensor("pixel_colors", [P, rows * K * 3], f32, kind="ExternalInput")
    out_d = nc.dram_tensor("out", [P, rows * 4], f32, kind="ExternalOutput")

    # const AP for the delta bias (EPS/GAMMA); framework pre-registers 0.0/1.0
    cbias = nc.alloc_sbuf_tensor("c_epsg", [P, 1], f32)
    nc.gpsimd.memset(cbias.ap(), EPS / GAMMA)
    nc.const_aps.aps[(f32, EPS / GAMMA)] = cbias.ap()
    nc.all_engine_barrier()

    with ExitStack() as ctx:
        sb = lambda name, w: ctx.enter_context(nc.sbuf_tensor(name, [P, w], f32))
        zb = [sb(f"zb{j}", TK) for j in range(2)]
        ds = [sb(f"ds{j}", TK) for j in range(2)]
        pf = [
            ctx.enter_context(nc.sbuf_tensor(f"pf{j}", [P, TK], dt.int32))
            for j in range(2)
        ]
        col = [sb(f"col{j}", TK * 3) for j in range(3)]
        ot = [sb(f"ot{j}", T * 4) for j in range(2)]
        sig = [sb(f"sig{j}", TK) for j in range(2)]    # prob, then w in place
        zlin = [sb(f"zlin{j}", TK) for j in range(2)]  # becomes zinv in place
        zd = sb("zd", TK)        # becomes ex in place (ACT)
        wc = sb("wc", TK * 3)
        zmax = sb("zmax", T)
        qsum = sb("qsum", T)
        wsum = sb("wsum", T)
        csum = sb("csum", T * 3)
        delta = [sb(f"delta{j}", T) for j in range(2)]   # cross-iter lifetime
        prodq = [sb(f"prodq{j}", T) for j in range(2)]   # cross-iter lifetime
        denom = sb("denom", T)
        rcp = sb("rcp", T)

        s_in = [
            ctx.enter_context(nc.semaphore("s_in0")),
            ctx.enter_context(nc.semaphore("s_in1")),
        ]
        s_out = [
            ctx.enter_context(nc.semaphore("s_out0")),
            ctx.enter_context(nc.semaphore("s_out1")),
        ]
        s_act = ctx.enter_context(nc.semaphore("s_act"))
        s_dve = ctx.enter_context(nc.semaphore("s_dve"))
        s_gp = ctx.enter_context(nc.semaphore("s_gp"))

        # ---- two-pass schedule: pass 1 records per-op sem values (marks),
        # ---- pass 2 emits with waits resolved from the marks.
        marks = {}

        def mk(engkey, name, t, ctr):
            marks[(engkey, name, t)] = ctr

        def sched_sp(sp):
            for i in range(n + 2):
                if i < n:
                    j = i % 2
                    if sp is not None:
                        if i >= 2:
                            sp.wait_ge(s_act, marks[("a", "zlin", i - 2)])
                            sp.wait_ge(s_dve, marks[("d", "qsum", i - 2)])
                        sp.dma_start(out=zb[j][:], in_=zb_d[:, bass.ts(i, TK)]
                                     ).then_inc(s_in[j], 16)
                        sp.dma_start(out=ds[j][:], in_=ds_d[:, bass.ts(i, TK)]
                                     ).then_inc(s_in[j], 16)
                        sp.dma_start(out=pf[j][:], in_=pf_d[:, bass.ts(i, TK)]
                                     ).then_inc(s_in[j], 16)
                        if i >= 3:
                            sp.wait_ge(s_dve, marks[("d", "wc", i - 3)])
                        sp.dma_start(out=col[i % 3][:], in_=pc_d[:, bass.ts(i, TK * 3)]
                                     ).then_inc(s_in[j], 16)
                if i >= 2 and i - 2 <= n - 1:
                    t = i - 2
                    if sp is not None:
                        sp.wait_ge(s_dve, marks[("d", "rgb", t)])
                        sp.wait_ge(s_act, marks[("a", "alpha", t)])
                        sp.dma_start(
                            out=out_d[:, bass.ts(t, T * 4)], in_=ot[t % 2][:]
                        ).then_inc(s_out[t % 2], 16)
            if sp is not None:
                sp.wait_ge(s_out[0], 16 * ((n + 1) // 2))
                sp.wait_ge(s_out[1], 16 * (n // 2))

        def sched_act(act):
            c = 0
            for i in range(n + 1):
                if i == 0:
                    if act is not None:
                        act.wait_ge(s_in[0], 64)
                        act.activation(sig[0][:], ds[0][:], Act.Sigmoid,
                                       scale=-1.0 / SIGMA).then_inc(s_act, 1)
                    c += 1; mk("a", "sig", 0, c)
                    if act is not None:
                        act.activation(
                            zlin[0][:], zb[0][:], Act.Copy,
                            bias=ZFAR / (ZFAR - ZNEAR),
                            scale=-1.0 / (ZFAR - ZNEAR),
                        ).then_inc(s_act, 1)
                    c += 1; mk("a", "zlin", 0, c)
                if i < n:
                    j = i % 2
                    if act is not None:
                        act.wait_ge(s_dve, marks[("d", "zinv", i)])
                        act.activation(pf[j][:].bitcast(f32), sig[j][:], Act.Ln,
                                       bias=1.0, scale=-1.0).then_inc(s_act, 1)
                    c += 1; mk("a", "lnq", i, c)
                    if act is not None:
                        act.wait_ge(s_dve, marks[("d", "zmax", i)])
                        act.activation(
                            delta[i % 2][:], zmax[:], Act.Exp,
                            bias=EPS / GAMMA, scale=-1.0 / GAMMA,
                        ).then_inc(s_act, 1)
                    c += 1; mk("a", "delta", i, c)
                    if act is not None:
                        act.wait_ge(s_dve, marks[("d", "qsum", i)])
                        act.activation(prodq[i % 2][:], qsum[:], Act.Exp
                                       ).then_inc(s_act, 1)
                    c += 1; mk("a", "prodq", i, c)
                if i >= 1:
                    t = i - 1
                    if act is not None:
                        act.wait_ge(s_dve, marks[("d", "denom", t)])
                        act.activation(denom[:], denom[:], Act.Ln
                                       ).then_inc(s_act, 1)
                        act.drain()
                    c += 1; mk("a", "lnd", t, c)
                    if act is not None:
                        act.activation(rcp[:], denom[:], Act.Exp, scale=-1.0
                                       ).then_inc(s_act, 1)
                    c += 1; mk("a", "rcp", t, c)
                    if act is not None:
                        if t >= 2:
                            act.wait_ge(s_out[t % 2], 16 * ((t - 2) // 2 + 1))
                        ot_v = ot[t % 2][:].rearrange("p (t q) -> p t q", q=4)
                        act.activation(
                            ot_v[:, :, 3:4], prodq[t % 2][:].unsqueeze(2),
                            Act.Copy, bias=1.0, scale=-1.0,
                        ).then_inc(s_act, 1)
                    c += 1; mk("a", "alpha", t, c)
                if i < n:
                    j = i % 2
                    if act is not None:
                        act.wait_ge(s_gp, marks[("g", "zd", i)])
                        act.activation(zd[:], zd[:], Act.Exp,
                                       scale=1.0 / GAMMA).then_inc(s_act, 1)
                    c += 1; mk("a", "ex", i, c)
                if i + 1 < n:
                    # next tile's sig/zlin emitted a full iteration early so
                    # the DVE A-phase never waits at the iter boundary
                    j2 = (i + 1) % 2
                    if act is not None:
                        act.wait_ge(s_in[j2], 64 * ((i + 1) // 2 + 1))
                        if i >= 1:
                            act.wait_ge(s_dve, marks[("d", "wsum", i - 1)])
                        act.activation(sig[j2][:], ds[j2][:], Act.Sigmoid,
                                       scale=-1.0 / SIGMA).then_inc(s_act, 1)
                    c += 1; mk("a", "sig", i + 1, c)
                    if act is not None:
                        act.activation(
                            zlin[j2][:], zb[j2][:], Act.Copy,
                            bias=ZFAR / (ZFAR - ZNEAR),
                            scale=-1.0 / (ZFAR - ZNEAR),
                        ).then_inc(s_act, 1)
                    c += 1; mk("a", "zlin", i + 1, c)

        def sched_dve(dve):
            c = 0
            for i in range(n + 1):
                if i < n:
                    j = i % 2
                    if dve is not None:
                        dve.wait_ge(s_in[j], 64 * (i // 2 + 1))
                        dve.wait_ge(s_act, marks[("a", "zlin", i)])
                        dve.scalar_tensor_tensor(
                            out=sig[j][:], in0=pf[j][:], scalar=0.0, in1=sig[j][:],
                            op0=Alu.is_ge, op1=Alu.mult,
                        ).then_inc(s_dve, 1)
                        dve.drain()
                    c += 1; mk("d", "prob", i, c)
                    if dve is not None:
                        dve.scalar_tensor_tensor(
                            out=zlin[j][:], in0=pf[j][:], scalar=0.0, in1=zlin[j][:],
                            op0=Alu.is_ge, op1=Alu.mult,
                        ).then_inc(s_dve, 1)
                        dve.drain()
                    c += 1; mk("d", "zinv", i, c)
                    if dve is not None:
                        dve.tensor_reduce(
                            out=zmax[:],
                            in_=zlin[j][:].rearrange("p (t k) -> p t k", k=K),
                            op=Alu.max, axis=Ax.X,
                        ).then_inc(s_dve, 1)
                        dve.drain()
                    c += 1; mk("d", "zmax", i, c)
                    if dve is not None:
                        dve.wait_ge(s_act, marks[("a", "lnq", i)])
                        dve.tensor_reduce(
                            out=qsum[:],
                            in_=pf[j][:].bitcast(f32)
                                .rearrange("p (t k) -> p t k", k=K),
                            op=Alu.add, axis=Ax.X,
                        ).then_inc(s_dve, 1)
                    c += 1; mk("d", "qsum", i, c)
                if i >= 1:
                    t = i - 1
                    if dve is not None:
                        dve.wait_ge(s_act, marks[("a", "ex", t)])
                        dve.tensor_tensor(
                            out=sig[t % 2][:], in0=sig[t % 2][:], in1=zd[:],
                            op=Alu.mult,
                        ).then_inc(s_dve, 1)
                        dve.drain()
                    c += 1; mk("d", "w", t, c)
                    if dve is not None:
                        dve.tensor_reduce(
                            out=wsum[:],
                            in_=sig[t % 2][:].rearrange("p (t k) -> p t k", k=K),
                            op=Alu.add, axis=Ax.X,
                        ).then_inc(s_dve, 1)
                        dve.drain()
                    c += 1; mk("d", "wsum", t, c)
                    if dve is not None:
                        dve.wait_ge(s_act, marks[("a", "delta", t)])
                        dve.tensor_tensor(
                            out=denom[:], in0=wsum[:], in1=delta[t % 2][:],
                            op=Alu.add,
                        ).then_inc(s_dve, 1)
                    c += 1; mk("d", "denom", t, c)
                    if dve is not None:
                        wc_v = wc[:].rearrange("p (t c k) -> p t c k", c=3, k=K)
                        dve.tensor_tensor(
                            out=wc_v,
                            in0=sig[t % 2][:].rearrange("p (t k) -> p t k", k=K)
                                .unsqueeze(2).broadcast_to((P, T, 3, K)),
                            in1=col[t % 3][:].rearrange(
                                "p (t k c) -> p t c k", k=K, c=3),
                            op=Alu.mult,
                        ).then_inc(s_dve, 1)
                        dve.drain()
                    c += 1; mk("d", "wc", t, c)
                    if dve is not None:
                        csum_v = csum[:].rearrange("p (t c) -> p t c", c=3)
                        dve.tensor_reduce(
                            out=csum_v, in_=wc_v, op=Alu.add, axis=Ax.X
                        ).then_inc(s_dve, 1)
                        dve.drain()
                    c += 1; mk("d", "csum", t, c)
                    if dve is not None:
                        csum_v = csum[:].rearrange("p (t c) -> p t c", c=3)
                        dve.tensor_tensor(
                            out=csum_v, in0=csum_v,
                            in1=delta[t % 2][:].unsqueeze(2).broadcast_to((P, T, 3)),
                            op=Alu.add,
                        ).then_inc(s_dve, 1)
                        dve.drain()
                    c += 1; mk("d", "t3", t, c)
                    if dve is not None:
                        if t >= 2:
                            dve.wait_ge(s_out[t % 2], 16 * ((t - 2) // 2 + 1))
                        dve.wait_ge(s_act, marks[("a", "rcp", t)])
                        ot_v = ot[t % 2][:].rearrange("p (t q) -> p t q", q=4)
                        dve.tensor_tensor(
                            out=ot_v[:, :, 0:3],
                            in0=csum[:].rearrange("p (t c) -> p t c", c=3),
                            in1=rcp[:].unsqueeze(2).broadcast_to((P, T, 3)),
                            op=Alu.mult,
                        ).then_inc(s_dve, 1)
                    c += 1; mk("d", "rgb", t, c)

        def sched_gp(gp):
            c = 0
            for i in range(n):
                j = i % 2
                if gp is not None:
                    if i >= 1:
                        # fires in the csum window (1-port on DVE) to dodge
                        # the shared-port lock; also covers zmax(i) + WAR w(i-1)
                        gp.wait_ge(s_dve, marks[("d", "wc", i - 1)])
                    else:
                        gp.wait_ge(s_dve, marks[("d", "zmax", i)])
                    gp.tensor_tensor(
                        out=zd[:].rearrange("p (t k) -> p t k", k=K),
                        in0=zlin[j][:].rearrange("p (t k) -> p t k", k=K),
                        in1=zmax[:].unsqueeze(2).broadcast_to((P, T, K)),
                        op=Alu.subtract,
                    ).then_inc(s_gp, 1)
                c += 1; mk("g", "zd", i, c)

        # pass 1: record marks
        sched_sp(None)
        sched_act(None)
        sched_dve(None)
        sched_gp(None)

        blk = ctx.enter_context(nc.Block())

        @blk.sync
        def _(sp):
            sched_sp(sp)

        @blk.scalar
        def _(act):
            sched_act(act)

        @blk.vector
        def _(dve):
            sched_dve(dve)

        @blk.gpsimd
        def _(gp):
            sched_gp(gp)

    return nc


_CACHE = {}


def _get_program(rows=2048, T=256):
    key = (rows, T)
    if key not in _CACHE:
        _CACHE[key] = build_program(rows, T)
    return _CACHE[key]


def _run(pixel_colors, zbuf, dists, pix_to_face, trace=False):
    from concourse.bass_utils import run_bass_kernel_spmd

    N, H, W, Kk = zbuf.shape
    assert (N, H, W, Kk) == (8, 512, 512, 8), (N, H, W, Kk)
    rows = H * W // P  # 2048

    nc = _get_program(rows=rows, T=256)

    pc = np.ascontiguousarray(np.asarray(pixel_colors, dtype=np.float32))
    zb = np.ascontiguousarray(np.asarray(zbuf, dtype=np.float32))
    ds = np.ascontiguousarray(np.asarray(dists, dtype=np.float32))
    pf = np.ascontiguousarray(np.asarray(pix_to_face, dtype=np.int32))

    in_maps = []
    for i in range(N_CORES):
        in_maps.append(
            {
                "zbuf": zb[i].reshape(P, rows * K),
                "dists": ds[i].reshape(P, rows * K),
                "pix_to_face": pf[i].reshape(P, rows * K),
                "pixel_colors": pc[i].reshape(P, rows * K * 3),
            }
        )

    res = run_bass_kernel_spmd(
        nc, in_maps, core_ids=list(range(N_CORES)), trace=trace
    )
    out = np.stack(
        [res.results[i]["out"].reshape(H, W, 4) for i in range(N_CORES)], axis=0
    )
    return out, res


def kernel(pixel_colors, zbuf, dists, pix_to_face):
    out, _ = _run(pixel_colors, zbuf, dists, pix_to_face, trace=False)
    return out



# revision 4
# speedup vs baseline: 1.5809x; 1.5809x over previous
"""Trainium2 Bass kernel for softmax RGB blend (pytorch3d NoLightShader).

Full inputs (N=8, H=512, W=512, K=8) are sharded batch-wise across 8
NeuronCores (one image per core); the blend is per-pixel, no cross-core
communication.

Host-side input encoding (per core):
    mask folded into the data (pix_to_face never shipped):
        d_eff = where(mask, dists, 1.0)        -> sigmoid(-d/SIGMA) = 0
        z_inv = (ZFAR - zbuf)/(ZFAR - ZNEAR) * mask
    z shipped as uint16 fixed point (z16 = round(65535 * z_inv)): u16 order
    matches float order, so the K-max runs in u16, and ACT's free affine
    (scale/bias) turns u16 straight into exp arguments.
    dists/colors shipped as bf16. Per-tile layout is k-major [K, T] (colors
    [3, K, T]) so every K-reduction is a contiguous pairwise fold tree at
    DVE 2x bf16 mode (tensor_reduce is stuck at 1x). One fused input DMA
    per tile: [z16 | d | col] = 20KB/partition lines. Output is planar
    bf16 [4, T] per tile (r|g|b|a), host transposes+upcasts.

Math per pixel:  p_k = sigmoid(-d_k/SIGMA); q_k = 1-p_k
    alpha = 1 - prod_k q_k                  (GPSIMD mult fold tree)
    zmax  = max_k z_k                       (DVE u16 max fold tree)
    w_k   = p_k * exp((z_k - zmax)/GAMMA)   (zd=zmax-z u16 on GPSIMD, exp ACT)
    delta = exp((EPS - zmax)/GAMMA)
    denom = sum_k w_k + delta               (DVE bf16 add fold tree)
    rgb   = (sum_k w_k c_k + delta) / denom (bg=1; wc + fold tree on DVE)
    out   = [rgb, alpha]

Engines: SP HWDGE DMAs (1 in + 1 out per tile) | ACT: sigmoid, exp(zd),
delta, ln(denom), rcp=exp(-ln), alpha (grouped to limit table switches) |
DVE: zmax folds, q, w, wsum folds, wc, csum folds, denom, t3, rgb |
GPSIMD: zd, prod-q folds. Raw bass with two-pass mark/wait scheduling
(single-sem waits), double-buffered tiles.
"""

import sys
from contextlib import ExitStack

import numpy as np

if "/opt/trn_rl_repo" not in sys.path:
    sys.path.insert(0, "/opt/trn_rl_repo")

SIGMA = 1e-4
GAMMA = 1e-4
ZNEAR = 1.0
ZFAR = 100.0
EPS = 1e-10

P = 128
K = 8
N_CORES = 8
ROWS = 2048          # H*W / P
T = 256              # pixels per partition per tile
NT = ROWS // T       # 8 tiles
TK = T * K           # 2048
IN_W = TK + TK + TK * 3   # u16 words per tile: z | d | col
OUT_W = T * 4             # bf16 words per tile (planar r|g|b|a)

S16G = (1.0 / 65535.0) / GAMMA   # u16 step -> 1/GAMMA units


def build_program():
    import concourse.bass as bass
    from concourse import mybir

    dt = mybir.dt
    f32 = dt.float32
    bf16 = dt.bfloat16
    u16 = dt.uint16
    Alu = mybir.AluOpType
    Act = mybir.ActivationFunctionType

    n = NT

    nc = bass.Bass()

    in_d = nc.dram_tensor("inb", [P, n * IN_W], u16, kind="ExternalInput")
    out_d = nc.dram_tensor("out", [P, n * OUT_W], u16, kind="ExternalOutput")

    # const AP for the delta bias (EPS/GAMMA); framework pre-registers 0.0/1.0
    cbias = nc.alloc_sbuf_tensor("c_epsg", [P, 1], f32)
    nc.gpsimd.memset(cbias.ap(), EPS / GAMMA)
    nc.const_aps.aps[(f32, EPS / GAMMA)] = cbias.ap()
    nc.all_engine_barrier()

    with ExitStack() as ctx:
        def sb(name, w, dty=bf16):
            return ctx.enter_context(nc.sbuf_tensor(name, [P, w], dty))

        inb = [sb(f"inb{j}", IN_W, u16) for j in range(2)]
        ot = [sb(f"ot{j}", OUT_W, u16) for j in range(2)]

        p_b = [sb(f"p{j}", TK) for j in range(2)]
        q_b = [sb(f"q{j}", TK) for j in range(2)]
        ex_b = [sb(f"ex{j}", TK) for j in range(2)]
        zd_b = [sb(f"zd{j}", TK, f32) for j in range(2)]
        zmax = [sb(f"zmax{j}", T, u16) for j in range(2)]
        delta = [sb(f"delta{j}", T, f32) for j in range(2)]
        prodq = [sb(f"prodq{j}", T) for j in range(2)]
        rcp = [sb(f"rcp{j}", T) for j in range(2)]
        t3b = [sb(f"t3{j}", T * 3) for j in range(2)]
        denom = [sb(f"denom{j}", T, f32) for j in range(2)]

        zm4 = sb("zm4", TK // 2, u16)
        zm2 = sb("zm2", TK // 4, u16)
        w_b = sb("w", TK)
        ws4 = sb("ws4", TK // 2)
        ws2 = sb("ws2", TK // 4)
        wsum = sb("wsum", T, f32)
        q4 = sb("q4", TK // 2)
        q2 = sb("q2", TK // 4)
        wc = sb("wc", TK * 3)
        cs4 = sb("cs4", TK * 3 // 2)
        cs2 = sb("cs2", TK * 3 // 4)
        csum = sb("csum", T * 3)
        lnden = sb("lnden", T, f32)

        s_in = [
            ctx.enter_context(nc.semaphore("s_in0")),
            ctx.enter_context(nc.semaphore("s_in1")),
        ]
        s_out = [
            ctx.enter_context(nc.semaphore("s_out0")),
            ctx.enter_context(nc.semaphore("s_out1")),
        ]
        s_act = ctx.enter_context(nc.semaphore("s_act"))
        s_dve = ctx.enter_context(nc.semaphore("s_dve"))
        s_gp = ctx.enter_context(nc.semaphore("s_gp"))

        marks = {}

        def mk(engkey, name, t, ctr):
            marks[(engkey, name, t)] = ctr

        # ---- SBUF views -------------------------------------------------
        def z_kt(j):      # [P, K, T] u16
            return inb[j][:, 0:TK].rearrange("p (k t) -> p k t", k=K)

        def d_bf(j):      # [P, TK] bf16
            return inb[j][:, TK:2 * TK].bitcast(bf16)

        def col_ckt(j):   # [P, 3, K, T] bf16
            return inb[j][:, 2 * TK:IN_W].bitcast(bf16).rearrange(
                "p (c k t) -> p c k t", c=3, k=K
            )

        def ot_rgb(j):    # [P, 3, T] bf16 planar
            return ot[j][:, 0:3 * T].bitcast(bf16).rearrange(
                "p (c t) -> p c t", c=3
            )

        def ot_a(j):      # [P, T] bf16
            return ot[j][:, 3 * T:4 * T].bitcast(bf16)

        # ---- schedules --------------------------------------------------
        def sched_sp(sp):
            for i in range(n + 2):
                if i < n:
                    j = i % 2
                    if sp is not None:
                        if i >= 2:
                            sp.wait_ge(s_act, marks[("a", "p", i - 2)])
                            sp.wait_ge(s_gp, marks[("g", "zd", i - 2)])
                            sp.wait_ge(s_dve, marks[("d", "wc", i - 2)])
                        sp.dma_start(
                            out=inb[j][:], in_=in_d[:, bass.ts(i, IN_W)]
                        ).then_inc(s_in[j], 16)
                if i >= 2:
                    u = i - 2
                    if sp is not None:
                        sp.wait_ge(s_dve, marks[("d", "rgb", u)])
                        sp.wait_ge(s_act, marks[("a", "alpha", u)])
                        sp.dma_start(
                            out=out_d[:, bass.ts(u, OUT_W)], in_=ot[u % 2][:]
                        ).then_inc(s_out[u % 2], 16)
            if sp is not None:
                sp.wait_ge(s_out[0], 16 * ((n + 1) // 2))
                sp.wait_ge(s_out[1], 16 * (n // 2))

        def sched_act(act):
            c = 0
            for i in range(n + 2):
                t = i - 1
                u = i - 2
                if i < n:
                    j = i % 2
                    if act is not None:
                        act.wait_ge(s_in[j], 16 * (i // 2 + 1))
                        if i >= 2:
                            act.wait_ge(s_dve, marks[("d", "w", i - 2)])
                        act.activation(
                            p_b[j][:], d_bf(j), Act.Sigmoid, scale=-1.0 / SIGMA
                        ).then_inc(s_act, 1)
                    c += 1; mk("a", "p", i, c)
                if u >= 0:
                    if act is not None:
                        act.wait_ge(s_dve, marks[("d", "denom", u)])
                        act.activation(lnden[:], denom[u % 2][:], Act.Ln
                                       ).then_inc(s_act, 1)
                    c += 1; mk("a", "lnd", u, c)
                    if act is not None:
                        if u >= 2:
                            act.wait_ge(s_dve, marks[("d", "rgb", u - 2)])
                        act.activation(rcp[u % 2][:], lnden[:], Act.Exp,
                                       scale=-1.0).then_inc(s_act, 1)
                    c += 1; mk("a", "rcp", u, c)
                if 0 <= t < n:
                    if act is not None:
                        act.wait_ge(s_gp, marks[("g", "zd", t)])
                        if t >= 2:
                            act.wait_ge(s_dve, marks[("d", "w", t - 2)])
                        act.activation(ex_b[t % 2][:], zd_b[t % 2][:], Act.Exp,
                                       scale=-S16G).then_inc(s_act, 1)
                    c += 1; mk("a", "ex", t, c)
                    if act is not None:
                        if t >= 2:
                            act.wait_ge(s_dve, marks[("d", "t3", t - 2)])
                        act.activation(
                            delta[t % 2][:], zmax[t % 2][:], Act.Exp,
                            bias=EPS / GAMMA, scale=-S16G,
                        ).then_inc(s_act, 1)
                    c += 1; mk("a", "delta", t, c)
                if u >= 0:
                    if act is not None:
                        act.wait_ge(s_gp, marks[("g", "qf3", u)])
                        if u >= 2:
                            act.wait_ge(s_out[u % 2], 16 * (u // 2))
                        act.activation(ot_a(u % 2), prodq[u % 2][:], Act.Copy,
                                       bias=1.0, scale=-1.0).then_inc(s_act, 1)
                    c += 1; mk("a", "alpha", u, c)

        def sched_dve(dve):
            c = 0
            for i in range(n + 2):
                t = i - 1
                u = i - 2
                if i < n:
                    j = i % 2
                    if dve is not None:
                        dve.wait_ge(s_in[j], 16 * (i // 2 + 1))
                        if i >= 2:
                            dve.wait_ge(s_gp, marks[("g", "zd", i - 2)])
                            dve.wait_ge(s_act, marks[("a", "delta", i - 2)])
                        zv = inb[j][:, 0:TK]
                        dve.tensor_tensor(
                            out=zm4[:], in0=zv[:, 0:TK // 2],
                            in1=zv[:, TK // 2:TK], op=Alu.max,
                        ).then_inc(s_dve, 1)
                    c += 1; mk("d", "zm1", i, c)
                    if dve is not None:
                        dve.tensor_tensor(
                            out=zm2[:], in0=zm4[:, 0:TK // 4],
                            in1=zm4[:, TK // 4:TK // 2], op=Alu.max,
                        ).then_inc(s_dve, 1)
                    c += 1; mk("d", "zm2", i, c)
                    if dve is not None:
                        dve.tensor_tensor(
                            out=zmax[j][:], in0=zm2[:, 0:T],
                            in1=zm2[:, T:2 * T], op=Alu.max,
                        ).then_inc(s_dve, 1)
                    c += 1; mk("d", "zm3", i, c)
                    if dve is not None:
                        dve.wait_ge(s_act, marks[("a", "p", i)])
                        if i >= 2:
                            dve.wait_ge(s_gp, marks[("g", "qf1", i - 2)])
                        dve.tensor_scalar(
                            out=q_b[j][:], in0=p_b[j][:], scalar1=-1.0,
                            scalar2=1.0, op0=Alu.mult, op1=Alu.add,
                        ).then_inc(s_dve, 1)
                    c += 1; mk("d", "q", i, c)
                if 0 <= t < n:
                    jt = t % 2
                    if dve is not None:
                        dve.wait_ge(s_act, marks[("a", "ex", t)])
                        dve.tensor_tensor(
                            out=w_b[:], in0=p_b[jt][:], in1=ex_b[jt][:],
                            op=Alu.mult,
                        ).then_inc(s_dve, 1)
                    c += 1; mk("d", "w", t, c)
                    if dve is not None:
                        dve.tensor_tensor(
                            out=ws4[:], in0=w_b[:, 0:TK // 2],
                            in1=w_b[:, TK // 2:TK], op=Alu.add,
                        ).then_inc(s_dve, 1)
                        dve.tensor_tensor(
                            out=ws2[:], in0=ws4[:, 0:TK // 4],
                            in1=ws4[:, TK // 4:TK // 2], op=Alu.add,
                        ).then_inc(s_dve, 1)
                        dve.tensor_tensor(
                            out=wsum[:], in0=ws2[:, 0:T],
                            in1=ws2[:, T:2 * T], op=Alu.add,
                        ).then_inc(s_dve, 1)
                    c += 3; mk("d", "wsum", t, c)
                    if dve is not None:
                        dve.wait_ge(s_act, marks[("a", "delta", t)])
                        dve.tensor_tensor(
                            out=denom[jt][:], in0=wsum[:], in1=delta[jt][:],
                            op=Alu.add,
                        ).then_inc(s_dve, 1)
                    c += 1; mk("d", "denom", t, c)
                    if dve is not None:
                        wv = w_b[:].rearrange("p (k t) -> p k t", k=K)
                        dve.tensor_tensor(
                            out=wc[:].rearrange("p (c k t) -> p c k t",
                                                c=3, k=K),
                            in0=wv.unsqueeze(1).broadcast_to((P, 3, K, T)),
                            in1=col_ckt(jt),
                            op=Alu.mult,
                        ).then_inc(s_dve, 1)
                    c += 1; mk("d", "wc", t, c)
                    if dve is not None:
                        wcv = wc[:].rearrange("p (c k t) -> p c k t", c=3, k=K)
                        dve.tensor_tensor(
                            out=cs4[:].rearrange("p (c k t) -> p c k t",
                                                 c=3, k=K // 2),
                            in0=wcv[:, :, 0:K // 2, :],
                            in1=wcv[:, :, K // 2:K, :], op=Alu.add,
                        ).then_inc(s_dve, 1)
                        cs4v = cs4[:].rearrange("p (c k t) -> p c k t",
                                                c=3, k=K // 2)
                        dve.tensor_tensor(
                            out=cs2[:].rearrange("p (c k t) -> p c k t",
                                                 c=3, k=K // 4),
                            in0=cs4v[:, :, 0:K // 4, :],
                            in1=cs4v[:, :, K // 4:K // 2, :], op=Alu.add,
                        ).then_inc(s_dve, 1)
                        cs2v = cs2[:].rearrange("p (c k t) -> p c k t",
                                                c=3, k=K // 4)
                        dve.tensor_tensor(
                            out=csum[:].rearrange("p (c t) -> p c t", c=3),
                            in0=cs2v[:, :, 0, :],
                            in1=cs2v[:, :, 1, :], op=Alu.add,
                        ).then_inc(s_dve, 1)
                    c += 3; mk("d", "csum", t, c)
                    if dve is not None:
                        dve.tensor_tensor(
                            out=t3b[jt][:].rearrange("p (c t) -> p c t", c=3),
                            in0=csum[:].rearrange("p (c t) -> p c t", c=3),
                            in1=delta[jt][:].unsqueeze(1).broadcast_to(
                                (P, 3, T)),
                            op=Alu.add,
                        ).then_inc(s_dve, 1)
                    c += 1; mk("d", "t3", t, c)
                if 0 <= u:
                    ju = u % 2
                    if dve is not None:
                        dve.wait_ge(s_act, marks[("a", "rcp", u)])
                        if u >= 2:
                            dve.wait_ge(s_out[ju], 16 * (u // 2))
                        dve.tensor_tensor(
                            out=ot_rgb(ju),
                            in0=t3b[ju][:].rearrange("p (c t) -> p c t", c=3),
                            in1=rcp[ju][:].unsqueeze(1).broadcast_to(
                                (P, 3, T)),
                            op=Alu.mult,
                        ).then_inc(s_dve, 1)
                    c += 1; mk("d", "rgb", u, c)

        def sched_gp(gp):
            c = 0
            for i in range(n):
                j = i % 2
                if gp is not None:
                    gp.wait_ge(s_dve, marks[("d", "zm3", i)])
                    if i >= 2:
                        gp.wait_ge(s_act, marks[("a", "ex", i - 2)])
                    gp.tensor_tensor(
                        out=zd_b[j][:].rearrange("p (k t) -> p k t", k=K),
                        in0=zmax[j][:].unsqueeze(1).broadcast_to((P, K, T)),
                        in1=z_kt(j),
                        op=Alu.subtract,
                    ).then_inc(s_gp, 1)
                c += 1; mk("g", "zd", i, c)
                if gp is not None:
                    gp.wait_ge(s_dve, marks[("d", "q", i)])
                    gp.tensor_tensor(
                        out=q4[:], in0=q_b[j][:, 0:TK // 2],
                        in1=q_b[j][:, TK // 2:TK], op=Alu.mult,
                    ).then_inc(s_gp, 1)
                c += 1; mk("g", "qf1", i, c)
                if gp is not None:
                    gp.tensor_tensor(
                        out=q2[:], in0=q4[:, 0:TK // 4],
                        in1=q4[:, TK // 4:TK // 2], op=Alu.mult,
                    ).then_inc(s_gp, 1)
                    if i >= 2:
                        gp.wait_ge(s_act, marks[("a", "alpha", i - 2)])
                    gp.tensor_tensor(
                        out=prodq[j][:], in0=q2[:, 0:T],
                        in1=q2[:, T:2 * T], op=Alu.mult,
                    ).then_inc(s_gp, 1)
                c += 2; mk("g", "qf3", i, c)

        # pass 1: record marks
        sched_sp(None)
        sched_act(None)
        sched_dve(None)
        sched_gp(None)

        blk = ctx.enter_context(nc.Block())

        @blk.sync
        def _(sp):
            sched_sp(sp)

        @blk.scalar
        def _(act):
            sched_act(act)

        @blk.vector
        def _(dve):
            sched_dve(dve)

        @blk.gpsimd
        def _(gp):
            sched_gp(gp)

    return nc


_CACHE = {}


def _get_program():
    if "nc" not in _CACHE:
        _CACHE["nc"] = build_program()
    return _CACHE["nc"]


def _pack_core(zb, ds, pf, pc, bf16_t):
    """Build the per-core [P, NT*IN_W] u16 input blob."""
    mask = pf >= 0
    z_inv = (ZFAR - zb) * (np.float32(1.0) / (ZFAR - ZNEAR))
    z_inv = np.where(mask, z_inv, np.float32(0.0))
    z16 = np.clip(np.rint(z_inv * np.float32(65535.0)), 0, 65535).astype(
        np.uint16
    )
    d_eff = np.where(mask, ds, np.float32(1.0)).astype(bf16_t).view(np.uint16)

    # pixel p-major: (H*W, K[,3]) -> [P, NT, ...] k-major tiles
    z16 = (
        z16.reshape(P, NT, T, K).transpose(0, 1, 3, 2).reshape(P, NT, TK)
    )
    d16 = (
        d_eff.reshape(P, NT, T, K).transpose(0, 1, 3, 2).reshape(P, NT, TK)
    )
    c16 = (
        pc.astype(bf16_t)
        .view(np.uint16)
        .reshape(P, NT, T, K, 3)
        .transpose(0, 1, 4, 3, 2)
        .reshape(P, NT, TK * 3)
    )
    return np.ascontiguousarray(
        np.concatenate([z16, d16, c16], axis=2)
    ).reshape(P, NT * IN_W)


def _run(pixel_colors, zbuf, dists, pix_to_face, trace=False):
    import ml_dtypes
    from concourse.bass_utils import run_bass_kernel_spmd

    bf16_t = ml_dtypes.bfloat16

    N, H, W, Kk = zbuf.shape
    assert (N, H, W, Kk) == (N_CORES, 512, 512, K), (N, H, W, Kk)

    nc = _get_program()

    pc = np.asarray(pixel_colors, dtype=np.float32)
    zb = np.asarray(zbuf, dtype=np.float32)
    ds = np.asarray(dists, dtype=np.float32)
    pf = np.asarray(pix_to_face)

    in_maps = []
    for i in range(N_CORES):
        blob = _pack_core(
            zb[i].reshape(-1, K),
            ds[i].reshape(-1, K),
            pf[i].reshape(-1, K),
            pc[i].reshape(-1, K, 3),
            bf16_t,
        )
        in_maps.append({"inb": blob})

    res = run_bass_kernel_spmd(
        nc, in_maps, core_ids=list(range(N_CORES)), trace=trace
    )
    outs = []
    for i in range(N_CORES):
        o = res.results[i]["out"]  # [P, NT*OUT_W] u16
        o = (
            np.ascontiguousarray(o)
            .view(bf16_t)
            .reshape(P, NT, 4, T)
            .transpose(0, 1, 3, 2)
            .astype(np.float32)
            .reshape(H, W, 4)
        )
        outs.append(o)
    return np.stack(outs, axis=0), res


def kernel(pixel_colors, zbuf, dists, pix_to_face):
    out, _ = _run(pixel_colors, zbuf, dists, pix_to_face, trace=False)
    return out


# revision 9
# speedup vs baseline: 1.6170x; 1.0229x over previous
"""Trainium2 Bass kernel for softmax RGB blend (pytorch3d NoLightShader).

Full inputs (N=8, H=512, W=512, K=8) are sharded batch-wise across 8
NeuronCores (one image per core); the blend is per-pixel, no cross-core
communication.

Host-side input encoding (per core):
    mask folded into the data (pix_to_face never shipped):
        d_eff = where(mask, dists, 1.0)        -> sigmoid(-d/SIGMA) = 0
        z_inv = (ZFAR - zbuf)/(ZFAR - ZNEAR) * mask
    z shipped as uint16 fixed point (z16 = round(65535 * z_inv)): u16 order
    matches float order, so the K-max runs in u16, and ACT's free affine
    (scale/bias) turns u16 straight into exp arguments.
    dists/colors shipped as bf16. Per-tile layout is k-major [K, T] (colors
    [3, K, T]) so every K-reduction is a contiguous pairwise fold tree at
    DVE 2x bf16 mode (tensor_reduce is stuck at 1x). dists ship as one
    up-front stream so ALL sigmoids run in a prepass -- the sigmoid and
    ln/exp ACT table sets otherwise swap twice per tile (~2.7us a load).
    Output is planar bf16 [4, T] per tile (r|g|b|a), host transposes.

Math per pixel:  p_k = sigmoid(-d_k/SIGMA); q_k = 1-p_k
    alpha = 1 - prod_k q_k     (DVE computes mq=p-1; GPSIMD mult fold tree;
                                8 negations cancel)
    zmax  = max_k z_k          (DVE u16 max fold tree)
    w_k   = p_k * exp((z_k - zmax)/GAMMA)  (zd=zmax-z fp16 on GPSIMD, exp ACT)
    delta = exp((EPS - zmax)/GAMMA)
    denom = sum_k w_k + delta              (DVE bf16 add fold tree)
    rgb   = (sum_k w_k c_k + delta)/denom  (bg=1; wc + fold tree on DVE)
    out   = [rgb, alpha]

Engines: SP HWDGE DMAs (d-stream + 1 in + 1 out per tile) | ACT: sigmoid
prepass, exp(zd), delta, ln(denom), rcp=exp(-ln), alpha | DVE: zmax folds,
mq, w, wc, wsum folds, denom, csum folds, t3, rgb | GPSIMD: zd, prod-q
folds. Raw bass, two-pass mark/wait scheduling, double-buffered tiles.
"""

import sys
from contextlib import ExitStack

import numpy as np

if "/opt/trn_rl_repo" not in sys.path:
    sys.path.insert(0, "/opt/trn_rl_repo")

SIGMA = 1e-4
GAMMA = 1e-4
ZNEAR = 1.0
ZFAR = 100.0
EPS = 1e-10

P = 128
K = 8
N_CORES = 8
ROWS = 2048          # H*W / P
T = 256              # pixels per partition per tile
NT = ROWS // T       # 8 tiles
TK = T * K           # 2048
IN_W = TK + TK * 3        # u16 words per tile: z | col
OUT_W = T * 4             # bf16 words per tile (planar r|g|b|a)

S16G = (1.0 / 65535.0) / GAMMA   # u16 step -> 1/GAMMA units


def build_program():
    import concourse.bass as bass
    from concourse import mybir

    dt = mybir.dt
    f32 = dt.float32
    bf16 = dt.bfloat16
    fp16 = dt.float16
    u16 = dt.uint16
    Alu = mybir.AluOpType
    Act = mybir.ActivationFunctionType

    n = NT

    nc = bass.Bass()

    in_d = nc.dram_tensor("inb", [P, n * IN_W], u16, kind="ExternalInput")
    d_d = nc.dram_tensor("din", [P, n * TK], u16, kind="ExternalInput")
    out_d = nc.dram_tensor("out", [P, n * OUT_W], u16, kind="ExternalOutput")

    # const AP for the delta bias (EPS/GAMMA); framework pre-registers 0.0/1.0
    cbias = nc.alloc_sbuf_tensor("c_epsg", [P, 1], f32)
    nc.gpsimd.memset(cbias.ap(), EPS / GAMMA)
    nc.const_aps.aps[(f32, EPS / GAMMA)] = cbias.ap()
    nc.all_engine_barrier()

    with ExitStack() as ctx:
        def sb(name, w, dty=bf16):
            return ctx.enter_context(nc.sbuf_tensor(name, [P, w], dty))

        inb = [sb(f"inb{j}", IN_W, u16) for j in range(2)]
        d_sb = sb("dall", n * TK, u16)
        p_all = sb("pall", n * TK)
        ot = [sb(f"ot{j}", OUT_W, u16) for j in range(2)]

        q_b = [sb(f"q{j}", TK) for j in range(2)]
        ex_b = [sb(f"ex{j}", TK) for j in range(2)]
        zd_b = [sb(f"zd{j}", TK, fp16) for j in range(2)]
        zmax = [sb(f"zmax{j}", T, u16) for j in range(2)]
        delta = [sb(f"delta{j}", T) for j in range(2)]
        prodq = [sb(f"prodq{j}", T) for j in range(2)]
        rcp = [sb(f"rcp{j}", T) for j in range(2)]
        t3b = [sb(f"t3{j}", T * 3) for j in range(2)]
        denom = [sb(f"denom{j}", T, f32) for j in range(2)]

        zm4 = sb("zm4", TK // 2, u16)
        zm2 = sb("zm2", TK // 4, u16)
        w_b = sb("w", TK)
        ws4 = sb("ws4", TK // 2)
        ws2 = sb("ws2", TK // 4)
        wsum = sb("wsum", T)
        q4 = sb("q4", TK // 2)
        q2 = sb("q2", TK // 4)
        wc = sb("wc", TK * 3)
        cs4 = sb("cs4", TK * 3 // 2)
        cs2 = sb("cs2", TK * 3 // 4)
        csum = sb("csum", T * 3)
        lnden = sb("lnden", T, f32)

        s_in = [
            ctx.enter_context(nc.semaphore("s_in0")),
            ctx.enter_context(nc.semaphore("s_in1")),
        ]
        s_out = [
            ctx.enter_context(nc.semaphore("s_out0")),
            ctx.enter_context(nc.semaphore("s_out1")),
        ]
        s_ind = ctx.enter_context(nc.semaphore("s_ind"))
        s_act = ctx.enter_context(nc.semaphore("s_act"))
        s_dve = ctx.enter_context(nc.semaphore("s_dve"))
        s_gp = ctx.enter_context(nc.semaphore("s_gp"))

        marks = {}

        def mk(engkey, name, t, ctr):
            marks[(engkey, name, t)] = ctr

        # ---- SBUF views -------------------------------------------------
        def z_kt(j):      # [P, K, T] u16
            return inb[j][:, 0:TK].rearrange("p (k t) -> p k t", k=K)

        def col_ckt(j):   # [P, 3, K, T] bf16
            return inb[j][:, TK:IN_W].bitcast(bf16).rearrange(
                "p (c k t) -> p c k t", c=3, k=K
            )

        def d_bf(i):      # [P, TK] bf16, tile i of the d stream
            return d_sb[:, bass.ts(i, TK)].bitcast(bf16)

        def p_t(i):       # [P, TK] bf16, tile i of the sigmoid prepass
            return p_all[:, bass.ts(i, TK)]

        def ot_rgb(j):    # [P, 3, T] bf16 planar
            return ot[j][:, 0:3 * T].bitcast(bf16).rearrange(
                "p (c t) -> p c t", c=3
            )

        def ot_a(j):      # [P, T] bf16
            return ot[j][:, 3 * T:4 * T].bitcast(bf16)

        # ---- schedules --------------------------------------------------
        def sched_sp(sp):
            if sp is not None:
                sp.dma_start(out=d_sb[:], in_=d_d[:, :]).then_inc(s_ind, 16)
            for i in range(n + 2):
                if i < n:
                    j = i % 2
                    if sp is not None:
                        if i >= 2:
                            sp.wait_ge(s_gp, marks[("g", "zd", i - 2)])
                            sp.wait_ge(s_dve, marks[("d", "wc", i - 2)])
                        sp.dma_start(
                            out=inb[j][:], in_=in_d[:, bass.ts(i, IN_W)]
                        ).then_inc(s_in[j], 16)
                if i >= 2:
                    u = i - 2
                    if sp is not None:
                        sp.wait_ge(s_dve, marks[("d", "rgb", u)])
                        sp.wait_ge(s_act, marks[("a", "alpha", u)])
                        sp.dma_start(
                            out=out_d[:, bass.ts(u, OUT_W)], in_=ot[u % 2][:]
                        ).then_inc(s_out[u % 2], 16)
            if sp is not None:
                sp.wait_ge(s_out[0], 16 * ((n + 1) // 2))
                sp.wait_ge(s_out[1], 16 * (n // 2))

        def sched_act(act):
            c = 0
            # sigmoid prepass: one table set, all tiles
            for i in range(n):
                if act is not None:
                    if i == 0:
                        act.wait_ge(s_ind, 16)
                    act.activation(
                        p_t(i), d_bf(i), Act.Sigmoid, scale=-1.0 / SIGMA
                    ).then_inc(s_act, 1)
                c += 1; mk("a", "p", i, c)
            for i in range(n + 2):
                t = i - 1
                u = i - 2
                if 0 <= t < n:
                    if act is not None:
                        act.wait_ge(s_gp, marks[("g", "zd", t)])
                        if t >= 2:
                            act.wait_ge(s_dve, marks[("d", "w", t - 2)])
                        act.activation(ex_b[t % 2][:], zd_b[t % 2][:], Act.Exp,
                                       scale=-S16G).then_inc(s_act, 1)
                    c += 1; mk("a", "ex", t, c)
                    if act is not None:
                        if t >= 2:
                            act.wait_ge(s_dve, marks[("d", "t3", t - 2)])
                        act.activation(
                            delta[t % 2][:], zmax[t % 2][:], Act.Exp,
                            bias=EPS / GAMMA, scale=-S16G,
                        ).then_inc(s_act, 1)
                    c += 1; mk("a", "delta", t, c)
                if u >= 0:
                    if act is not None:
                        act.wait_ge(s_dve, marks[("d", "denom", u)])
                        act.activation(lnden[:], denom[u % 2][:], Act.Ln
                                       ).then_inc(s_act, 1)
                    c += 1; mk("a", "lnd", u, c)
                    if act is not None:
                        if u >= 2:
                            act.wait_ge(s_dve, marks[("d", "rgb", u - 2)])
                        act.activation(rcp[u % 2][:], lnden[:], Act.Exp,
                                       scale=-1.0).then_inc(s_act, 1)
                    c += 1; mk("a", "rcp", u, c)
                    if act is not None:
                        act.wait_ge(s_gp, marks[("g", "qf3", u)])
                        if u >= 2:
                            act.wait_ge(s_out[u % 2], 16 * (u // 2))
                        act.activation(ot_a(u % 2), prodq[u % 2][:], Act.Copy,
                                       bias=1.0, scale=-1.0).then_inc(s_act, 1)
                    c += 1; mk("a", "alpha", u, c)

        def sched_dve(dve):
            c = 0
            for i in range(n + 2):
                t = i - 1
                u = i - 2
                if i < n:
                    j = i % 2
                    if dve is not None:
                        dve.wait_ge(s_in[j], 16 * (i // 2 + 1))
                        if i >= 2:
                            dve.wait_ge(s_gp, marks[("g", "zd", i - 2)])
                            dve.wait_ge(s_act, marks[("a", "delta", i - 2)])
                        zv = inb[j][:, 0:TK]
                        dve.tensor_tensor(
                            out=zm4[:], in0=zv[:, 0:TK // 2],
                            in1=zv[:, TK // 2:TK], op=Alu.max,
                        ).then_inc(s_dve, 1)
                    c += 1; mk("d", "zm1", i, c)
                    if dve is not None:
                        dve.tensor_tensor(
                            out=zm2[:], in0=zm4[:, 0:TK // 4],
                            in1=zm4[:, TK // 4:TK // 2], op=Alu.max,
                        ).then_inc(s_dve, 1)
                    c += 1; mk("d", "zm2", i, c)
                    if dve is not None:
                        dve.tensor_tensor(
                            out=zmax[j][:], in0=zm2[:, 0:T],
                            in1=zm2[:, T:2 * T], op=Alu.max,
                        ).then_inc(s_dve, 1)
                    c += 1; mk("d", "zm3", i, c)
                    if dve is not None:
                        dve.wait_ge(s_act, marks[("a", "p", i)])
                        if i >= 2:
                            dve.wait_ge(s_gp, marks[("g", "qf1", i - 2)])
                        # mq = p - 1 = -q; the 8-way product cancels signs
                        dve.tensor_scalar(
                            out=q_b[j][:], in0=p_t(i), scalar1=1.0,
                            scalar2=None, op0=Alu.subtract,
                        ).then_inc(s_dve, 1)
                    c += 1; mk("d", "q", i, c)
                if 0 <= t < n:
                    jt = t % 2
                    if dve is not None:
                        dve.wait_ge(s_act, marks[("a", "ex", t)])
                        dve.tensor_tensor(
                            out=w_b[:], in0=p_t(t), in1=ex_b[jt][:],
                            op=Alu.mult,
                        ).then_inc(s_dve, 1)
                    c += 1; mk("d", "w", t, c)
                    if dve is not None:
                        wv = w_b[:].rearrange("p (k t) -> p k t", k=K)
                        dve.tensor_tensor(
                            out=wc[:].rearrange("p (c k t) -> p c k t",
                                                c=3, k=K),
                            in0=col_ckt(jt),
                            in1=wv.unsqueeze(1).broadcast_to((P, 3, K, T)),
                            op=Alu.mult,
                        ).then_inc(s_dve, 1)
                    c += 1; mk("d", "wc", t, c)
                    if dve is not None:
                        dve.tensor_tensor(
                            out=ws4[:], in0=w_b[:, 0:TK // 2],
                            in1=w_b[:, TK // 2:TK], op=Alu.add,
                        ).then_inc(s_dve, 1)
                        dve.tensor_tensor(
                            out=ws2[:], in0=ws4[:, 0:TK // 4],
                            in1=ws4[:, TK // 4:TK // 2], op=Alu.add,
                        ).then_inc(s_dve, 1)
                        dve.tensor_tensor(
                            out=wsum[:], in0=ws2[:, 0:T],
                            in1=ws2[:, T:2 * T], op=Alu.add,
                        ).then_inc(s_dve, 1)
                    c += 3; mk("d", "wsum", t, c)
                    if dve is not None:
                        dve.wait_ge(s_act, marks[("a", "delta", t)])
                        dve.tensor_tensor(
                            out=denom[jt][:], in0=wsum[:], in1=delta[jt][:],
                            op=Alu.add,
                        ).then_inc(s_dve, 1)
                    c += 1; mk("d", "denom", t, c)
                    if dve is not None:
                        wcv = wc[:].rearrange("p (c k t) -> p c k t", c=3, k=K)
                        dve.tensor_tensor(
                            out=cs4[:].rearrange("p (c k t) -> p c k t",
                                                 c=3, k=K // 2),
                            in0=wcv[:, :, 0:K // 2, :],
                            in1=wcv[:, :, K // 2:K, :], op=Alu.add,
                        ).then_inc(s_dve, 1)
                        cs4v = cs4[:].rearrange("p (c k t) -> p c k t",
                                                c=3, k=K // 2)
                        dve.tensor_tensor(
                            out=cs2[:].rearrange("p (c k t) -> p c k t",
                                                 c=3, k=K // 4),
                            in0=cs4v[:, :, 0:K // 4, :],
                            in1=cs4v[:, :, K // 4:K // 2, :], op=Alu.add,
                        ).then_inc(s_dve, 1)
                        cs2v = cs2[:].rearrange("p (c k t) -> p c k t",
                                                c=3, k=K // 4)
                        dve.tensor_tensor(
                            out=csum[:].rearrange("p (c t) -> p c t", c=3),
                            in0=cs2v[:, :, 0, :],
                            in1=cs2v[:, :, 1, :], op=Alu.add,
                        ).then_inc(s_dve, 1)
                    c += 3; mk("d", "csum", t, c)
                    if dve is not None:
                        dve.tensor_tensor(
                            out=t3b[jt][:].rearrange("p (c t) -> p c t", c=3),
                            in0=csum[:].rearrange("p (c t) -> p c t", c=3),
                            in1=delta[jt][:].unsqueeze(1).broadcast_to(
                                (P, 3, T)),
                            op=Alu.add,
                        ).then_inc(s_dve, 1)
                    c += 1; mk("d", "t3", t, c)
                if 0 <= u:
                    ju = u % 2
                    if dve is not None:
                        dve.wait_ge(s_act, marks[("a", "rcp", u)])
                        if u >= 2:
                            dve.wait_ge(s_out[ju], 16 * (u // 2))
                        dve.tensor_tensor(
                            out=ot_rgb(ju),
                            in0=t3b[ju][:].rearrange("p (c t) -> p c t", c=3),
                            in1=rcp[ju][:].unsqueeze(1).broadcast_to(
                                (P, 3, T)),
                            op=Alu.mult,
                        ).then_inc(s_dve, 1)
                    c += 1; mk("d", "rgb", u, c)

        def sched_gp(gp):
            c = 0
            for i in range(n):
                j = i % 2
                if gp is not None:
                    gp.wait_ge(s_dve, marks[("d", "zm3", i)])
                    if i >= 2:
                        gp.wait_ge(s_act, marks[("a", "ex", i - 2)])
                    gp.tensor_tensor(
                        out=zd_b[j][:].rearrange("p (k t) -> p k t", k=K),
                        in0=zmax[j][:].unsqueeze(1).broadcast_to((P, K, T)),
                        in1=z_kt(j),
                        op=Alu.subtract,
                    ).then_inc(s_gp, 1)
                c += 1; mk("g", "zd", i, c)
                if gp is not None:
                    gp.wait_ge(s_dve, marks[("d", "q", i)])
                    gp.tensor_tensor(
                        out=q4[:], in0=q_b[j][:, 0:TK // 2],
                        in1=q_b[j][:, TK // 2:TK], op=Alu.mult,
                    ).then_inc(s_gp, 1)
                c += 1; mk("g", "qf1", i, c)
                if gp is not None:
                    gp.tensor_tensor(
                        out=q2[:], in0=q4[:, 0:TK // 4],
                        in1=q4[:, TK // 4:TK // 2], op=Alu.mult,
                    ).then_inc(s_gp, 1)
                    if i >= 2:
                        gp.wait_ge(s_act, marks[("a", "alpha", i - 2)])
                    gp.tensor_tensor(
                        out=prodq[j][:], in0=q2[:, 0:T],
                        in1=q2[:, T:2 * T], op=Alu.mult,
                    ).then_inc(s_gp, 1)
                c += 2; mk("g", "qf3", i, c)

        # pass 1: record marks
        sched_sp(None)
        sched_act(None)
        sched_dve(None)
        sched_gp(None)

        blk = ctx.enter_context(nc.Block())

        @blk.sync
        def _(sp):
            sched_sp(sp)

        @blk.scalar
        def _(act):
            sched_act(act)

        @blk.vector
        def _(dve):
            sched_dve(dve)

        @blk.gpsimd
        def _(gp):
            sched_gp(gp)

    return nc


_CACHE = {}


def _get_program():
    if "nc" not in _CACHE:
        _CACHE["nc"] = build_program()
    return _CACHE["nc"]


def _pack_core(zb, ds, pf, pc, bf16_t):
    """Per-core input: [P, NT*IN_W] u16 blob (z|col) and [P, NT*TK] d."""
    mask = pf >= 0
    z_inv = (ZFAR - zb) * (np.float32(1.0) / (ZFAR - ZNEAR))
    z_inv = np.where(mask, z_inv, np.float32(0.0))
    z16 = np.clip(np.rint(z_inv * np.float32(65535.0)), 0, 65535).astype(
        np.uint16
    )
    d_eff = np.where(mask, ds, np.float32(1.0)).astype(bf16_t).view(np.uint16)

    # pixel p-major: (H*W, K[,3]) -> [P, NT, ...] k-major tiles
    z16 = (
        z16.reshape(P, NT, T, K).transpose(0, 1, 3, 2).reshape(P, NT, TK)
    )
    d16 = (
        d_eff.reshape(P, NT, T, K).transpose(0, 1, 3, 2).reshape(P, NT * TK)
    )
    c16 = (
        pc.astype(bf16_t)
        .view(np.uint16)
        .reshape(P, NT, T, K, 3)
        .transpose(0, 1, 4, 3, 2)
        .reshape(P, NT, TK * 3)
    )
    blob = np.ascontiguousarray(
        np.concatenate([z16, c16], axis=2)
    ).reshape(P, NT * IN_W)
    return blob, np.ascontiguousarray(d16)


def _run(pixel_colors, zbuf, dists, pix_to_face, trace=False):
    import ml_dtypes
    from concourse.bass_utils import run_bass_kernel_spmd

    bf16_t = ml_dtypes.bfloat16

    N, H, W, Kk = zbuf.shape
    assert (N, H, W, Kk) == (N_CORES, 512, 512, K), (N, H, W, Kk)

    nc = _get_program()

    pc = np.asarray(pixel_colors, dtype=np.float32)
    zb = np.asarray(zbuf, dtype=np.float32)
    ds = np.asarray(dists, dtype=np.float32)
    pf = np.asarray(pix_to_face)

    in_maps = []
    for i in range(N_CORES):
        blob, din = _pack_core(
            zb[i].reshape(-1, K),
            ds[i].reshape(-1, K),
            pf[i].reshape(-1, K),
            pc[i].reshape(-1, K, 3),
            bf16_t,
        )
        in_maps.append({"inb": blob, "din": din})

    res = run_bass_kernel_spmd(
        nc, in_maps, core_ids=list(range(N_CORES)), trace=trace
    )
    outs = []
    for i in range(N_CORES):
        o = res.results[i]["out"]  # [P, NT*OUT_W] u16
        o = (
            np.ascontiguousarray(o)
            .view(bf16_t)
            .reshape(P, NT, 4, T)
            .transpose(0, 1, 3, 2)
            .astype(np.float32)
            .reshape(H, W, 4)
        )
        outs.append(o)
    return np.stack(outs, axis=0), res


def kernel(pixel_colors, zbuf, dists, pix_to_face):
    out, _ = _run(pixel_colors, zbuf, dists, pix_to_face, trace=False)
    return out


# revision 20
# speedup vs baseline: 1.8640x; 1.1527x over previous
"""Trainium2 Bass kernel for softmax RGB blend (pytorch3d NoLightShader).

Full inputs (N=8, H=512, W=512, K=8) are sharded batch-wise across 8
NeuronCores (one image per core); the blend is per-pixel, no cross-core
communication.

Host-side input encoding (per core):
    mask folded into the data (pix_to_face never shipped):
        d_eff = where(mask, dists, 1.0)        -> sigmoid(-d/SIGMA) = 0
        z_inv = (ZFAR - zbuf)/(ZFAR - ZNEAR) * mask
    z shipped as uint16 fixed point (z16 = round(65535 * z_inv)): u16 order
    matches float order, so the K-max runs in u16, and ACT's free affine
    (scale/bias) turns u16 straight into exp arguments.
    dists/colors shipped as bf16. Per-tile layout is k-major [K, T] (colors
    [3, K, T]) so every K-reduction is a contiguous pairwise fold tree at
    DVE 2x bf16 mode (tensor_reduce is stuck at 1x). dists ship as one
    up-front stream so ALL sigmoids run in a prepass -- the sigmoid and
    ln/exp ACT table sets otherwise swap twice per tile (~2.7us a load).
    Output is planar bf16 [4, T] per tile (r|g|b|a), host transposes.

Math per pixel:  p_k = sigmoid(-d_k/SIGMA); q_k = 1-p_k
    alpha = 1 - prod_k q_k     (DVE computes mq=p-1; GPSIMD mult fold tree;
                                8 negations cancel)
    zmax  = max_k z_k          (DVE u16 max fold tree)
    w_k   = p_k * exp((z_k - zmax)/GAMMA)  (zd=zmax-z fp16 on GPSIMD, exp ACT)
    delta = exp((EPS - zmax)/GAMMA)
    denom = sum_k w_k + delta              (DVE bf16 add fold tree)
    rgb   = (sum_k w_k c_k + delta)/denom  (bg=1; wc + fold tree on DVE)
    out   = [rgb, alpha]

Engines: SP HWDGE DMAs (d-stream + 1 in + 1 out per tile) | ACT: sigmoid
prepass, exp(zd), delta, ln(denom), rcp=exp(-ln), alpha | DVE: zmax folds,
mq, w, wc, wsum folds, denom, csum folds, t3, rgb | GPSIMD: zd, prod-q
folds. Raw bass, two-pass mark/wait scheduling, double-buffered tiles.
"""

import sys
from contextlib import ExitStack

import numpy as np

if "/opt/trn_rl_repo" not in sys.path:
    sys.path.insert(0, "/opt/trn_rl_repo")

SIGMA = 1e-4
GAMMA = 1e-4
ZNEAR = 1.0
ZFAR = 100.0
EPS = 1e-10

P = 128
K = 8
N_CORES = 8
ROWS = 2048          # H*W / P
T = 256              # pixels per partition per tile
NT = ROWS // T       # 8 tiles
TK = T * K           # 2048
IN_W = TK + TK * 3        # u16 words per tile: z | col
OUT_W = T * 4             # bf16 words per tile (planar r|g|b|a)

S16G = (1.0 / 65535.0) / GAMMA   # u16 step -> 1/GAMMA units


def build_program():
    import concourse.bass as bass
    from concourse import mybir

    dt = mybir.dt
    f32 = dt.float32
    bf16 = dt.bfloat16
    fp16 = dt.float16
    u16 = dt.uint16
    Alu = mybir.AluOpType
    Act = mybir.ActivationFunctionType

    n = NT

    nc = bass.Bass()

    in_d = nc.dram_tensor("inb", [P, n * IN_W], u16, kind="ExternalInput")
    d_d = nc.dram_tensor("din", [P, n * TK], u16, kind="ExternalInput")
    out_d = nc.dram_tensor("out", [P, n * OUT_W], u16, kind="ExternalOutput")

    # const AP for the delta bias (EPS/GAMMA); framework pre-registers 0.0/1.0
    cbias = nc.alloc_sbuf_tensor("c_epsg", [P, 1], f32)
    nc.gpsimd.memset(cbias.ap(), EPS / GAMMA)
    nc.const_aps.aps[(f32, EPS / GAMMA)] = cbias.ap()
    nc.all_engine_barrier()

    with ExitStack() as ctx:
        def sb(name, w, dty=bf16):
            return ctx.enter_context(nc.sbuf_tensor(name, [P, w], dty))

        inb = [sb(f"inb{j}", IN_W, u16) for j in range(2)]
        d_sb = sb("dall", n * TK, u16)
        p_all = sb("pall", n * TK)
        ot = [sb(f"ot{j}", OUT_W, u16) for j in range(2)]

        q_b = [sb(f"q{j}", TK) for j in range(2)]
        ex_b = [sb(f"ex{j}", TK) for j in range(2)]
        zd_b = [sb(f"zd{j}", TK, fp16) for j in range(2)]
        zmax = [sb(f"zmax{j}", T, u16) for j in range(2)]
        delta = [sb(f"delta{j}", T) for j in range(2)]
        prodq = [sb(f"prodq{j}", T) for j in range(2)]
        rcp = [sb(f"rcp{j}", T) for j in range(2)]
        t3b = [sb(f"t3{j}", T * 3) for j in range(2)]
        denom = [sb(f"denom{j}", T, f32) for j in range(2)]

        zm4 = sb("zm4", TK // 2, u16)
        zm2 = sb("zm2", TK // 4, u16)
        w_b = sb("w", TK)
        ws4 = sb("ws4", TK // 2)
        ws2 = sb("ws2", TK // 4)
        wsum = sb("wsum", T)
        q4 = sb("q4", TK // 2)
        q2 = sb("q2", TK // 4)
        wc = sb("wc", TK * 3)
        cs4 = sb("cs4", TK * 3 // 2)
        cs2 = sb("cs2", TK * 3 // 4)
        csum = sb("csum", T * 3)
        lnden = sb("lnden", T, f32)

        s_in = [
            ctx.enter_context(nc.semaphore("s_in0")),
            ctx.enter_context(nc.semaphore("s_in1")),
        ]
        s_out = [
            ctx.enter_context(nc.semaphore("s_out0")),
            ctx.enter_context(nc.semaphore("s_out1")),
        ]
        s_ind = ctx.enter_context(nc.semaphore("s_ind"))
        s_act = ctx.enter_context(nc.semaphore("s_act"))
        s_dve = ctx.enter_context(nc.semaphore("s_dve"))
        s_gp = ctx.enter_context(nc.semaphore("s_gp"))

        marks = {}

        def mk(engkey, name, t, ctr):
            marks[(engkey, name, t)] = ctr

        # ---- SBUF views -------------------------------------------------
        def z_kt(j):      # [P, K, T] u16
            return inb[j][:, 0:TK].rearrange("p (k t) -> p k t", k=K)

        def col_ckt(j):   # [P, 3, K, T] bf16
            return inb[j][:, TK:IN_W].bitcast(bf16).rearrange(
                "p (c k t) -> p c k t", c=3, k=K
            )

        def d_bf(i):      # [P, TK] bf16, tile i of the d stream
            return d_sb[:, bass.ts(i, TK)].bitcast(bf16)

        def p_t(i):       # [P, TK] bf16, tile i of the sigmoid prepass
            return p_all[:, bass.ts(i, TK)]

        def ot_rgb(j):    # [P, 3, T] bf16 planar
            return ot[j][:, 0:3 * T].bitcast(bf16).rearrange(
                "p (c t) -> p c t", c=3
            )

        def ot_a(j):      # [P, T] bf16
            return ot[j][:, 3 * T:4 * T].bitcast(bf16)

        # ---- schedules --------------------------------------------------
        def sched_sp(sp):
            if sp is not None:
                sp.dma_start(out=d_sb[:], in_=d_d[:, :]).then_inc(s_ind, 16)
            for i in range(n + 2):
                if i < n:
                    j = i % 2
                    if sp is not None:
                        if i >= 2:
                            sp.wait_ge(s_dve, marks[("d", "wc", i - 2)])
                        sp.dma_start(
                            out=inb[j][:], in_=in_d[:, bass.ts(i, IN_W)]
                        ).then_inc(s_in[j], 16)
                if i >= 2:
                    u = i - 2
                    if sp is not None:
                        sp.wait_ge(s_dve, marks[("d", "rgb", u)])
                        sp.wait_ge(s_act, marks[("a", "alpha", u)])
                        sp.dma_start(
                            out=out_d[:, bass.ts(u, OUT_W)], in_=ot[u % 2][:]
                        ).then_inc(s_out[u % 2], 16)
            if sp is not None:
                sp.wait_ge(s_out[0], 16 * ((n + 1) // 2))
                sp.wait_ge(s_out[1], 16 * (n // 2))

        def sched_act(act):
            c = 0
            # sigmoid prepass: one table set, all tiles
            for i in range(n):
                if act is not None:
                    if i == 0:
                        act.wait_ge(s_ind, 16)
                    act.activation(
                        p_t(i), d_bf(i), Act.Sigmoid, scale=-1.0 / SIGMA
                    ).then_inc(s_act, 1)
                c += 1; mk("a", "p", i, c)
            for i in range(n + 2):
                t = i - 1
                u = i - 2
                if 0 <= t < n:
                    if act is not None:
                        act.wait_ge(s_dve, marks[("d", "zd", t)])
                        if t >= 2:
                            act.wait_ge(s_dve, marks[("d", "w", t - 2)])
                        act.activation(ex_b[t % 2][:], zd_b[t % 2][:], Act.Exp,
                                       scale=S16G).then_inc(s_act, 1)
                    c += 1; mk("a", "ex", t, c)
                    if act is not None:
                        if t >= 2:
                            act.wait_ge(s_dve, marks[("d", "t3", t - 2)])
                        act.activation(
                            delta[t % 2][:], zmax[t % 2][:], Act.Exp,
                            bias=EPS / GAMMA, scale=-S16G,
                        ).then_inc(s_act, 1)
                    c += 1; mk("a", "delta", t, c)
                if u >= 0:
                    if act is not None:
                        act.wait_ge(s_dve, marks[("d", "denom", u)])
                        act.activation(lnden[:], denom[u % 2][:], Act.Ln
                                       ).then_inc(s_act, 1)
                    c += 1; mk("a", "lnd", u, c)
                    if act is not None:
                        if u >= 2:
                            act.wait_ge(s_dve, marks[("d", "rgb", u - 2)])
                        act.activation(rcp[u % 2][:], lnden[:], Act.Exp,
                                       scale=-1.0).then_inc(s_act, 1)
                    c += 1; mk("a", "rcp", u, c)
                    if act is not None:
                        act.wait_ge(s_dve, marks[("d", "qf3", u)])
                        if u >= 2:
                            act.wait_ge(s_out[u % 2], 16 * (u // 2))
                        act.activation(ot_a(u % 2), prodq[u % 2][:], Act.Copy,
                                       bias=1.0, scale=-1.0).then_inc(s_act, 1)
                    c += 1; mk("a", "alpha", u, c)

        def sched_dve(dve):
            c = 0
            for i in range(n + 2):
                t = i - 1
                u = i - 2
                if i < n:
                    j = i % 2
                    if dve is not None:
                        dve.wait_ge(s_in[j], 16 * (i // 2 + 1))
                        if i >= 2:
                            dve.wait_ge(s_act, marks[("a", "delta", i - 2)])
                        zv = inb[j][:, 0:TK]
                        dve.tensor_tensor(
                            out=zm4[:], in0=zv[:, 0:TK // 2],
                            in1=zv[:, TK // 2:TK], op=Alu.max,
                        ).then_inc(s_dve, 1)
                    c += 1; mk("d", "zm1", i, c)
                    if dve is not None:
                        dve.tensor_tensor(
                            out=zm2[:], in0=zm4[:, 0:TK // 4],
                            in1=zm4[:, TK // 4:TK // 2], op=Alu.max,
                        ).then_inc(s_dve, 1)
                    c += 1; mk("d", "zm2", i, c)
                    if dve is not None:
                        dve.tensor_tensor(
                            out=zmax[j][:], in0=zm2[:, 0:T],
                            in1=zm2[:, T:2 * T], op=Alu.max,
                        ).then_inc(s_dve, 1)
                    c += 1; mk("d", "zm3", i, c)
                    if dve is not None:
                        if i >= 2:
                            dve.wait_ge(s_act, marks[("a", "ex", i - 2)])
                        dve.tensor_tensor(
                            out=zd_b[j][:].rearrange("p (k t) -> p k t", k=K),
                            in0=z_kt(j),
                            in1=zmax[j][:].unsqueeze(1).broadcast_to(
                                (P, K, T)),
                            op=Alu.subtract,
                        ).then_inc(s_dve, 1)
                    c += 1; mk("d", "zd", i, c)
                    if dve is not None:
                        dve.wait_ge(s_act, marks[("a", "p", i)])
                        # mq = p - 1 = -q; the 8-way product cancels signs
                        dve.tensor_scalar(
                            out=q_b[j][:], in0=p_t(i), scalar1=1.0,
                            scalar2=None, op0=Alu.subtract,
                        ).then_inc(s_dve, 1)
                    c += 1; mk("d", "q", i, c)
                    if dve is not None:
                        dve.tensor_tensor(
                            out=q4[:], in0=q_b[j][:, 0:TK // 2],
                            in1=q_b[j][:, TK // 2:TK], op=Alu.mult,
                        ).then_inc(s_dve, 1)
                        dve.tensor_tensor(
                            out=q2[:], in0=q4[:, 0:TK // 4],
                            in1=q4[:, TK // 4:TK // 2], op=Alu.mult,
                        ).then_inc(s_dve, 1)
                        if i >= 2:
                            dve.wait_ge(s_act, marks[("a", "alpha", i - 2)])
                        dve.tensor_tensor(
                            out=prodq[j][:], in0=q2[:, 0:T],
                            in1=q2[:, T:2 * T], op=Alu.mult,
                        ).then_inc(s_dve, 1)
                    c += 3; mk("d", "qf3", i, c)
                if 0 <= t < n:
                    jt = t % 2
                    if dve is not None:
                        dve.wait_ge(s_act, marks[("a", "ex", t)])
                        dve.tensor_tensor(
                            out=w_b[:], in0=p_t(t), in1=ex_b[jt][:],
                            op=Alu.mult,
                        ).then_inc(s_dve, 1)
                    c += 1; mk("d", "w", t, c)
                    if dve is not None:
                        wv = w_b[:].rearrange("p (k t) -> p k t", k=K)
                        dve.tensor_tensor(
                            out=wc[:].rearrange("p (c k t) -> p c k t",
                                                c=3, k=K),
                            in0=col_ckt(jt),
                            in1=wv.unsqueeze(1).broadcast_to((P, 3, K, T)),
                            op=Alu.mult,
                        ).then_inc(s_dve, 1)
                    c += 1; mk("d", "wc", t, c)
                    if dve is not None:
                        dve.tensor_tensor(
                            out=ws4[:], in0=w_b[:, 0:TK // 2],
                            in1=w_b[:, TK // 2:TK], op=Alu.add,
                        ).then_inc(s_dve, 1)
                        dve.tensor_tensor(
                            out=ws2[:], in0=ws4[:, 0:TK // 4],
                            in1=ws4[:, TK // 4:TK // 2], op=Alu.add,
                        ).then_inc(s_dve, 1)
                        dve.tensor_tensor(
                            out=wsum[:], in0=ws2[:, 0:T],
                            in1=ws2[:, T:2 * T], op=Alu.add,
                        ).then_inc(s_dve, 1)
                    c += 3; mk("d", "wsum", t, c)
                    if dve is not None:
                        dve.wait_ge(s_act, marks[("a", "delta", t)])
                        dve.tensor_tensor(
                            out=denom[jt][:], in0=wsum[:], in1=delta[jt][:],
                            op=Alu.add,
                        ).then_inc(s_dve, 1)
                    c += 1; mk("d", "denom", t, c)
                    if dve is not None:
                        wcv = wc[:].rearrange("p (c k t) -> p c k t", c=3, k=K)
                        dve.tensor_tensor(
                            out=cs4[:].rearrange("p (c k t) -> p c k t",
                                                 c=3, k=K // 2),
                            in0=wcv[:, :, 0:K // 2, :],
                            in1=wcv[:, :, K // 2:K, :], op=Alu.add,
                        ).then_inc(s_dve, 1)
                        cs4v = cs4[:].rearrange("p (c k t) -> p c k t",
                                                c=3, k=K // 2)
                        dve.tensor_tensor(
                            out=cs2[:].rearrange("p (c k t) -> p c k t",
                                                 c=3, k=K // 4),
                            in0=cs4v[:, :, 0:K // 4, :],
                            in1=cs4v[:, :, K // 4:K // 2, :], op=Alu.add,
                        ).then_inc(s_dve, 1)
                        cs2v = cs2[:].rearrange("p (c k t) -> p c k t",
                                                c=3, k=K // 4)
                        dve.tensor_tensor(
                            out=csum[:].rearrange("p (c t) -> p c t", c=3),
                            in0=cs2v[:, :, 0, :],
                            in1=cs2v[:, :, 1, :], op=Alu.add,
                        ).then_inc(s_dve, 1)
                    c += 3; mk("d", "csum", t, c)
                    if dve is not None:
                        dve.tensor_tensor(
                            out=t3b[jt][:].rearrange("p (c t) -> p c t", c=3),
                            in0=csum[:].rearrange("p (c t) -> p c t", c=3),
                            in1=delta[jt][:].unsqueeze(1).broadcast_to(
                                (P, 3, T)),
                            op=Alu.add,
                        ).then_inc(s_dve, 1)
                    c += 1; mk("d", "t3", t, c)
                if 0 <= u:
                    ju = u % 2
                    if dve is not None:
                        dve.wait_ge(s_act, marks[("a", "rcp", u)])
                        if u >= 2:
                            dve.wait_ge(s_out[ju], 16 * (u // 2))
                        dve.tensor_tensor(
                            out=ot_rgb(ju),
                            in0=t3b[ju][:].rearrange("p (c t) -> p c t", c=3),
                            in1=rcp[ju][:].unsqueeze(1).broadcast_to(
                                (P, 3, T)),
                            op=Alu.mult,
                        ).then_inc(s_dve, 1)
                    c += 1; mk("d", "rgb", u, c)

        # pass 1: record marks
        sched_sp(None)
        sched_act(None)
        sched_dve(None)

        blk = ctx.enter_context(nc.Block())

        @blk.sync
        def _(sp):
            sched_sp(sp)

        @blk.scalar
        def _(act):
            sched_act(act)

        @blk.vector
        def _(dve):
            sched_dve(dve)

    return nc


_CACHE = {}


def _get_program():
    if "nc" not in _CACHE:
        _CACHE["nc"] = build_program()
    return _CACHE["nc"]


def _pack_core(zb, ds, pf, pc, bf16_t):
    """Per-core input: [P, NT*IN_W] u16 blob (z|col) and [P, NT*TK] d."""
    mask = pf >= 0
    z_inv = (ZFAR - zb) * (np.float32(1.0) / (ZFAR - ZNEAR))
    z_inv = np.where(mask, z_inv, np.float32(0.0))
    z16 = np.clip(np.rint(z_inv * np.float32(65535.0)), 0, 65535).astype(
        np.uint16
    )
    d_eff = np.where(mask, ds, np.float32(1.0)).astype(bf16_t).view(np.uint16)

    # pixel p-major: (H*W, K[,3]) -> [P, NT, ...] k-major tiles
    z16 = (
        z16.reshape(P, NT, T, K).transpose(0, 1, 3, 2).reshape(P, NT, TK)
    )
    d16 = (
        d_eff.reshape(P, NT, T, K).transpose(0, 1, 3, 2).reshape(P, NT * TK)
    )
    c16 = (
        pc.astype(bf16_t)
        .view(np.uint16)
        .reshape(P, NT, T, K, 3)
        .transpose(0, 1, 4, 3, 2)
        .reshape(P, NT, TK * 3)
    )
    blob = np.ascontiguousarray(
        np.concatenate([z16, c16], axis=2)
    ).reshape(P, NT * IN_W)
    return blob, np.ascontiguousarray(d16)


def _run(pixel_colors, zbuf, dists, pix_to_face, trace=False):
    import ml_dtypes
    from concourse.bass_utils import run_bass_kernel_spmd

    bf16_t = ml_dtypes.bfloat16

    N, H, W, Kk = zbuf.shape
    assert (N, H, W, Kk) == (N_CORES, 512, 512, K), (N, H, W, Kk)

    nc = _get_program()

    pc = np.asarray(pixel_colors, dtype=np.float32)
    zb = np.asarray(zbuf, dtype=np.float32)
    ds = np.asarray(dists, dtype=np.float32)
    pf = np.asarray(pix_to_face)

    in_maps = []
    for i in range(N_CORES):
        blob, din = _pack_core(
            zb[i].reshape(-1, K),
            ds[i].reshape(-1, K),
            pf[i].reshape(-1, K),
            pc[i].reshape(-1, K, 3),
            bf16_t,
        )
        in_maps.append({"inb": blob, "din": din})

    res = run_bass_kernel_spmd(
        nc, in_maps, core_ids=list(range(N_CORES)), trace=trace
    )
    outs = []
    for i in range(N_CORES):
        o = res.results[i]["out"]  # [P, NT*OUT_W] u16
        o = (
            np.ascontiguousarray(o)
            .view(bf16_t)
            .reshape(P, NT, 4, T)
            .transpose(0, 1, 3, 2)
            .astype(np.float32)
            .reshape(H, W, 4)
        )
        outs.append(o)
    return np.stack(outs, axis=0), res


def kernel(pixel_colors, zbuf, dists, pix_to_face):
    out, _ = _run(pixel_colors, zbuf, dists, pix_to_face, trace=False)
    return out


# revision 27
# speedup vs baseline: 1.9527x; 1.0476x over previous
"""Trainium2 Bass kernel for softmax RGB blend (pytorch3d NoLightShader).

Full inputs (N=8, H=512, W=512, K=8) are sharded batch-wise across 8
NeuronCores (one image per core); the blend is per-pixel, no cross-core
communication.

Host-side input encoding (per core):
    mask folded into the data (pix_to_face never shipped):
        d_eff = where(mask, dists, 1.0)        -> sigmoid(-d/SIGMA) = 0
        z_inv = (ZFAR - zbuf)/(ZFAR - ZNEAR) * mask
    z shipped as uint16 fixed point (z16 = round(65535 * z_inv)): u16 order
    matches float order, so the K-max runs in u16, and ACT's free affine
    (scale/bias) turns u16 straight into exp arguments.
    dists/colors shipped as bf16. Per-tile layout is k-major [K, T] (colors
    [3, K, T]) so every K-reduction is a contiguous pairwise fold tree at
    DVE 2x bf16 mode (tensor_reduce is stuck at 1x). dists ship as one
    up-front stream so ALL sigmoids run in a prepass -- the sigmoid and
    ln/exp ACT table sets otherwise swap twice per tile (~2.7us a load).
    Output is planar bf16 [4, T] per tile (r|g|b|a), host transposes.

Math per pixel:  p_k = sigmoid(-d_k/SIGMA); q_k = 1-p_k
    alpha = 1 - prod_k q_k     (DVE computes mq=p-1; GPSIMD mult fold tree;
                                8 negations cancel)
    zmax  = max_k z_k          (DVE u16 max fold tree)
    w_k   = p_k * exp((z_k - zmax)/GAMMA)  (zd=zmax-z fp16 on GPSIMD, exp ACT)
    delta = exp((EPS - zmax)/GAMMA)
    denom = sum_k w_k + delta              (DVE bf16 add fold tree)
    rgb   = (sum_k w_k c_k + delta)/denom  (bg=1; wc + fold tree on DVE)
    out   = [rgb, alpha]

Engines: SP HWDGE DMAs (d-stream + 1 in + 1 out per tile) | ACT: sigmoid
prepass, exp(zd), delta, ln(denom), rcp=exp(-ln), alpha | DVE: zmax folds,
mq, w, wc, wsum folds, denom, csum folds, t3, rgb | GPSIMD: zd, prod-q
folds. Raw bass, two-pass mark/wait scheduling, double-buffered tiles.
"""

import sys
from contextlib import ExitStack

import numpy as np

if "/opt/trn_rl_repo" not in sys.path:
    sys.path.insert(0, "/opt/trn_rl_repo")

SIGMA = 1e-4
GAMMA = 1e-4
ZNEAR = 1.0
ZFAR = 100.0
EPS = 1e-10

P = 128
K = 8
N_CORES = 8
ROWS = 2048          # H*W / P
T = 256              # pixels per partition per tile
NT = ROWS // T       # 8 tiles
TK = T * K           # 2048
IN_W = TK + TK * 3        # u16 words per tile: z | col
OUT_W = T * 4             # bf16 words per tile (planar r|g|b|a)

S16G = (1.0 / 65535.0) / GAMMA   # u16 step -> 1/GAMMA units


def build_program():
    import concourse.bass as bass
    from concourse import mybir

    dt = mybir.dt
    f32 = dt.float32
    bf16 = dt.bfloat16
    fp16 = dt.float16
    u16 = dt.uint16
    Alu = mybir.AluOpType
    Act = mybir.ActivationFunctionType

    n = NT

    nc = bass.Bass()

    in_d = nc.dram_tensor("inb", [P, n * IN_W], u16, kind="ExternalInput")
    d_d = nc.dram_tensor("din", [P, n * TK], u16, kind="ExternalInput")
    out_d = nc.dram_tensor("out", [P, n * OUT_W], u16, kind="ExternalOutput")

    # const AP for the delta bias (EPS/GAMMA); framework pre-registers 0.0/1.0
    cbias = nc.alloc_sbuf_tensor("c_epsg", [P, 1], f32)
    nc.gpsimd.memset(cbias.ap(), EPS / GAMMA)
    nc.const_aps.aps[(f32, EPS / GAMMA)] = cbias.ap()
    nc.all_engine_barrier()

    with ExitStack() as ctx:
        def sb(name, w, dty=bf16):
            return ctx.enter_context(nc.sbuf_tensor(name, [P, w], dty))

        NB = 3  # input tile buffers
        inb = [sb(f"inb{j}", IN_W, u16) for j in range(NB)]
        d_sb = sb("dall", n * TK, u16)   # d bf16; sigmoid overwrites in place
        ot = [sb(f"ot{j}", OUT_W, u16) for j in range(2)]

        q_b = [sb(f"q{j}", TK) for j in range(2)]
        ex_b = [sb(f"ex{j}", TK) for j in range(2)]
        zd_b = [sb(f"zd{j}", TK, fp16) for j in range(2)]
        zmax = [sb(f"zmax{j}", T, u16) for j in range(2)]
        delta = [sb(f"delta{j}", T) for j in range(2)]
        prodq = [sb(f"prodq{j}", T) for j in range(2)]
        rcp = [sb(f"rcp{j}", T) for j in range(2)]
        t3b = [sb(f"t3{j}", T * 3) for j in range(2)]
        denom = [sb(f"denom{j}", T, f32) for j in range(2)]

        zm4 = sb("zm4", TK // 2, u16)
        zm2 = sb("zm2", TK // 4, u16)
        w_b = sb("w", TK)
        ws4 = sb("ws4", TK // 2)
        ws2 = sb("ws2", TK // 4)
        wsum = sb("wsum", T)
        q4 = sb("q4", TK // 2)
        q2 = sb("q2", TK // 4)
        wc = sb("wc", TK * 3)
        cs4 = sb("cs4", TK * 3 // 2)
        cs2 = sb("cs2", TK * 3 // 4)
        csum = sb("csum", T * 3)
        lnden = sb("lnden", T, f32)

        s_in = [
            ctx.enter_context(nc.semaphore("s_in0")),
            ctx.enter_context(nc.semaphore("s_in1")),
            ctx.enter_context(nc.semaphore("s_in2")),
        ]
        s_out = [
            ctx.enter_context(nc.semaphore("s_out0")),
            ctx.enter_context(nc.semaphore("s_out1")),
        ]
        s_ind = ctx.enter_context(nc.semaphore("s_ind"))
        s_act = ctx.enter_context(nc.semaphore("s_act"))
        s_dve = ctx.enter_context(nc.semaphore("s_dve"))
        s_gp = ctx.enter_context(nc.semaphore("s_gp"))

        marks = {}

        def mk(engkey, name, t, ctr):
            marks[(engkey, name, t)] = ctr

        # ---- SBUF views -------------------------------------------------
        def z_kt(j):      # [P, K, T] u16
            return inb[j][:, 0:TK].rearrange("p (k t) -> p k t", k=K)

        def col_ckt(j):   # [P, 3, K, T] bf16
            return inb[j][:, TK:IN_W].bitcast(bf16).rearrange(
                "p (c k t) -> p c k t", c=3, k=K
            )

        def d_bf(i):      # [P, TK] bf16, tile i of the d stream
            return d_sb[:, bass.ts(i, TK)].bitcast(bf16)

        def p_t(i):       # [P, TK] bf16, tile i of sigmoid (in-place over d)
            return d_sb[:, bass.ts(i, TK)].bitcast(bf16)

        def ot_rgb(j):    # [P, 3, T] bf16 planar
            return ot[j][:, 0:3 * T].bitcast(bf16).rearrange(
                "p (c t) -> p c t", c=3
            )

        def ot_a(j):      # [P, T] bf16
            return ot[j][:, 3 * T:4 * T].bitcast(bf16)

        # ---- schedules --------------------------------------------------
        def sched_sp(sp):
            if sp is not None:
                sp.dma_start(out=d_sb[:], in_=d_d[:, :]).then_inc(s_ind, 16)
            for i in range(n + 2):
                if i < n:
                    j = i % NB
                    if sp is not None:
                        if i >= NB:
                            sp.wait_ge(s_dve, marks[("d", "wc", i - NB)])
                        sp.dma_start(
                            out=inb[j][:], in_=in_d[:, bass.ts(i, IN_W)]
                        ).then_inc(s_in[j], 16)
                if i >= 2:
                    u = i - 2
                    if sp is not None:
                        sp.wait_ge(s_dve, marks[("d", "rgb", u)])
                        sp.wait_ge(s_act, marks[("a", "alpha", u)])
                        sp.dma_start(
                            out=out_d[:, bass.ts(u, OUT_W)], in_=ot[u % 2][:]
                        ).then_inc(s_out[u % 2], 16)
            if sp is not None:
                sp.wait_ge(s_out[0], 16 * ((n + 1) // 2))
                sp.wait_ge(s_out[1], 16 * (n // 2))

        def sched_act(act):
            c = 0
            # sigmoid prepass: one table set, all tiles
            for i in range(n):
                if act is not None:
                    if i == 0:
                        act.wait_ge(s_ind, 16)
                    act.activation(
                        p_t(i), d_bf(i), Act.Sigmoid, scale=-1.0 / SIGMA
                    ).then_inc(s_act, 1)
                c += 1; mk("a", "p", i, c)
            for i in range(n + 2):
                t = i - 1
                u = i - 2
                if 0 <= t < n:
                    if act is not None:
                        act.wait_ge(s_dve, marks[("d", "zd", t)])
                        if t >= 2:
                            act.wait_ge(s_dve, marks[("d", "w", t - 2)])
                        act.activation(ex_b[t % 2][:], zd_b[t % 2][:], Act.Exp,
                                       scale=S16G).then_inc(s_act, 1)
                    c += 1; mk("a", "ex", t, c)
                    if act is not None:
                        if t >= 2:
                            act.wait_ge(s_dve, marks[("d", "t3", t - 2)])
                        act.activation(
                            delta[t % 2][:], zmax[t % 2][:], Act.Exp,
                            bias=EPS / GAMMA, scale=-S16G,
                        ).then_inc(s_act, 1)
                    c += 1; mk("a", "delta", t, c)
                if u >= 0:
                    if act is not None:
                        act.wait_ge(s_dve, marks[("d", "denom", u)])
                        act.activation(lnden[:], denom[u % 2][:], Act.Ln
                                       ).then_inc(s_act, 1)
                    c += 1; mk("a", "lnd", u, c)
                    if act is not None:
                        if u >= 2:
                            act.wait_ge(s_dve, marks[("d", "rgb", u - 2)])
                        act.activation(rcp[u % 2][:], lnden[:], Act.Exp,
                                       scale=-1.0).then_inc(s_act, 1)
                    c += 1; mk("a", "rcp", u, c)
                    if act is not None:
                        act.wait_ge(s_dve, marks[("d", "qf3", u)])
                        if u >= 2:
                            act.wait_ge(s_out[u % 2], 16 * (u // 2))
                        act.activation(ot_a(u % 2), prodq[u % 2][:], Act.Copy,
                                       bias=1.0, scale=-1.0).then_inc(s_act, 1)
                    c += 1; mk("a", "alpha", u, c)

        def sched_dve(dve):
            c = 0
            for i in range(n + 2):
                t = i - 1
                u = i - 2
                if i < n:
                    j = i % 2
                    jb = i % NB
                    if dve is not None:
                        dve.wait_ge(s_in[jb], 16 * (i // NB + 1))
                        if i >= 2:
                            dve.wait_ge(s_act, marks[("a", "delta", i - 2)])
                        zv = inb[jb][:, 0:TK]
                        dve.tensor_tensor(
                            out=zm4[:], in0=zv[:, 0:TK // 2],
                            in1=zv[:, TK // 2:TK], op=Alu.max,
                        ).then_inc(s_dve, 1)
                    c += 1; mk("d", "zm1", i, c)
                    if dve is not None:
                        dve.tensor_tensor(
                            out=zm2[:], in0=zm4[:, 0:TK // 4],
                            in1=zm4[:, TK // 4:TK // 2], op=Alu.max,
                        ).then_inc(s_dve, 1)
                    c += 1; mk("d", "zm2", i, c)
                    if dve is not None:
                        dve.tensor_tensor(
                            out=zmax[j][:], in0=zm2[:, 0:T],
                            in1=zm2[:, T:2 * T], op=Alu.max,
                        ).then_inc(s_dve, 1)
                    c += 1; mk("d", "zm3", i, c)
                    if dve is not None:
                        if i >= 2:
                            dve.wait_ge(s_act, marks[("a", "ex", i - 2)])
                        dve.tensor_tensor(
                            out=zd_b[j][:].rearrange("p (k t) -> p k t", k=K),
                            in0=z_kt(jb),
                            in1=zmax[j][:].unsqueeze(1).broadcast_to(
                                (P, K, T)),
                            op=Alu.subtract,
                        ).then_inc(s_dve, 1)
                    c += 1; mk("d", "zd", i, c)
                    if dve is not None:
                        dve.wait_ge(s_act, marks[("a", "p", i)])
                        # mq = p - 1 = -q; the 8-way product cancels signs
                        dve.tensor_scalar(
                            out=q_b[j][:], in0=p_t(i), scalar1=1.0,
                            scalar2=None, op0=Alu.subtract,
                        ).then_inc(s_dve, 1)
                    c += 1; mk("d", "q", i, c)
                    if dve is not None:
                        dve.tensor_tensor(
                            out=q4[:], in0=q_b[j][:, 0:TK // 2],
                            in1=q_b[j][:, TK // 2:TK], op=Alu.mult,
                        ).then_inc(s_dve, 1)
                        dve.tensor_tensor(
                            out=q2[:], in0=q4[:, 0:TK // 4],
                            in1=q4[:, TK // 4:TK // 2], op=Alu.mult,
                        ).then_inc(s_dve, 1)
                        if i >= 2:
                            dve.wait_ge(s_act, marks[("a", "alpha", i - 2)])
                        dve.tensor_tensor(
                            out=prodq[j][:], in0=q2[:, 0:T],
                            in1=q2[:, T:2 * T], op=Alu.mult,
                        ).then_inc(s_dve, 1)
                    c += 3; mk("d", "qf3", i, c)
                if 0 <= t < n:
                    jt = t % 2
                    jtb = t % NB
                    if dve is not None:
                        dve.wait_ge(s_act, marks[("a", "ex", t)])
                        dve.tensor_tensor(
                            out=w_b[:], in0=p_t(t), in1=ex_b[jt][:],
                            op=Alu.mult,
                        ).then_inc(s_dve, 1)
                    c += 1; mk("d", "w", t, c)
                    if dve is not None:
                        dve.tensor_tensor(
                            out=wc[:].rearrange("p (c kt) -> p c kt", c=3),
                            in0=inb[jtb][:, TK:IN_W].bitcast(bf16).rearrange(
                                "p (c kt) -> p c kt", c=3),
                            in1=w_b[:].unsqueeze(1).broadcast_to((P, 3, TK)),
                            op=Alu.mult,
                        ).then_inc(s_dve, 1)
                    c += 1; mk("d", "wc", t, c)
                    if dve is not None:
                        dve.tensor_tensor(
                            out=ws4[:], in0=w_b[:, 0:TK // 2],
                            in1=w_b[:, TK // 2:TK], op=Alu.add,
                        ).then_inc(s_dve, 1)
                        dve.tensor_tensor(
                            out=ws2[:], in0=ws4[:, 0:TK // 4],
                            in1=ws4[:, TK // 4:TK // 2], op=Alu.add,
                        ).then_inc(s_dve, 1)
                        dve.tensor_tensor(
                            out=wsum[:], in0=ws2[:, 0:T],
                            in1=ws2[:, T:2 * T], op=Alu.add,
                        ).then_inc(s_dve, 1)
                    c += 3; mk("d", "wsum", t, c)
                    if dve is not None:
                        dve.wait_ge(s_act, marks[("a", "delta", t)])
                        dve.tensor_tensor(
                            out=denom[jt][:], in0=wsum[:], in1=delta[jt][:],
                            op=Alu.add,
                        ).then_inc(s_dve, 1)
                    c += 1; mk("d", "denom", t, c)
                    if dve is not None:
                        wcv = wc[:].rearrange("p (c k t) -> p c k t", c=3, k=K)
                        dve.tensor_tensor(
                            out=cs4[:].rearrange("p (c k t) -> p c k t",
                                                 c=3, k=K // 2),
                            in0=wcv[:, :, 0:K // 2, :],
                            in1=wcv[:, :, K // 2:K, :], op=Alu.add,
                        ).then_inc(s_dve, 1)
                        cs4v = cs4[:].rearrange("p (c k t) -> p c k t",
                                                c=3, k=K // 2)
                        dve.tensor_tensor(
                            out=cs2[:].rearrange("p (c k t) -> p c k t",
                                                 c=3, k=K // 4),
                            in0=cs4v[:, :, 0:K // 4, :],
                            in1=cs4v[:, :, K // 4:K // 2, :], op=Alu.add,
                        ).then_inc(s_dve, 1)
                        cs2v = cs2[:].rearrange("p (c k t) -> p c k t",
                                                c=3, k=K // 4)
                        dve.tensor_tensor(
                            out=csum[:].rearrange("p (c t) -> p c t", c=3),
                            in0=cs2v[:, :, 0, :],
                            in1=cs2v[:, :, 1, :], op=Alu.add,
                        ).then_inc(s_dve, 1)
                    c += 3; mk("d", "csum", t, c)
                    if dve is not None:
                        dve.tensor_tensor(
                            out=t3b[jt][:].rearrange("p (c t) -> p c t", c=3),
                            in0=csum[:].rearrange("p (c t) -> p c t", c=3),
                            in1=delta[jt][:].unsqueeze(1).broadcast_to(
                                (P, 3, T)),
                            op=Alu.add,
                        ).then_inc(s_dve, 1)
                    c += 1; mk("d", "t3", t, c)
                if 0 <= u:
                    ju = u % 2
                    if dve is not None:
                        dve.wait_ge(s_act, marks[("a", "rcp", u)])
                        if u >= 2:
                            dve.wait_ge(s_out[ju], 16 * (u // 2))
                        dve.tensor_tensor(
                            out=ot_rgb(ju),
                            in0=t3b[ju][:].rearrange("p (c t) -> p c t", c=3),
                            in1=rcp[ju][:].unsqueeze(1).broadcast_to(
                                (P, 3, T)),
                            op=Alu.mult,
                        ).then_inc(s_dve, 1)
                    c += 1; mk("d", "rgb", u, c)

        # pass 1: record marks
        sched_sp(None)
        sched_act(None)
        sched_dve(None)

        blk = ctx.enter_context(nc.Block())

        @blk.sync
        def _(sp):
            sched_sp(sp)

        @blk.scalar
        def _(act):
            sched_act(act)

        @blk.vector
        def _(dve):
            sched_dve(dve)

    return nc


_CACHE = {}


def _get_program():
    if "nc" not in _CACHE:
        _CACHE["nc"] = build_program()
    return _CACHE["nc"]


def _pack_core(zb, ds, pf, pc, bf16_t):
    """Per-core input: [P, NT*IN_W] u16 blob (z|col) and [P, NT*TK] d."""
    mask = pf >= 0
    z_inv = (ZFAR - zb) * (np.float32(1.0) / (ZFAR - ZNEAR))
    z_inv = np.where(mask, z_inv, np.float32(0.0))
    z16 = np.clip(np.rint(z_inv * np.float32(65535.0)), 0, 65535).astype(
        np.uint16
    )
    d_eff = np.where(mask, ds, np.float32(1.0)).astype(bf16_t).view(np.uint16)

    # pixel p-major: (H*W, K[,3]) -> [P, NT, ...] k-major tiles
    z16 = (
        z16.reshape(P, NT, T, K).transpose(0, 1, 3, 2).reshape(P, NT, TK)
    )
    d16 = (
        d_eff.reshape(P, NT, T, K).transpose(0, 1, 3, 2).reshape(P, NT * TK)
    )
    c16 = (
        pc.astype(bf16_t)
        .view(np.uint16)
        .reshape(P, NT, T, K, 3)
        .transpose(0, 1, 4, 3, 2)
        .reshape(P, NT, TK * 3)
    )
    blob = np.ascontiguousarray(
        np.concatenate([z16, c16], axis=2)
    ).reshape(P, NT * IN_W)
    return blob, np.ascontiguousarray(d16)


def _run(pixel_colors, zbuf, dists, pix_to_face, trace=False):
    import ml_dtypes
    from concourse.bass_utils import run_bass_kernel_spmd

    bf16_t = ml_dtypes.bfloat16

    N, H, W, Kk = zbuf.shape
    assert (N, H, W, Kk) == (N_CORES, 512, 512, K), (N, H, W, Kk)

    nc = _get_program()

    pc = np.asarray(pixel_colors, dtype=np.float32)
    zb = np.asarray(zbuf, dtype=np.float32)
    ds = np.asarray(dists, dtype=np.float32)
    pf = np.asarray(pix_to_face)

    in_maps = []
    for i in range(N_CORES):
        blob, din = _pack_core(
            zb[i].reshape(-1, K),
            ds[i].reshape(-1, K),
            pf[i].reshape(-1, K),
            pc[i].reshape(-1, K, 3),
            bf16_t,
        )
        in_maps.append({"inb": blob, "din": din})

    res = run_bass_kernel_spmd(
        nc, in_maps, core_ids=list(range(N_CORES)), trace=trace
    )
    outs = []
    for i in range(N_CORES):
        o = res.results[i]["out"]  # [P, NT*OUT_W] u16
        o = (
            np.ascontiguousarray(o)
            .view(bf16_t)
            .reshape(P, NT, 4, T)
            .transpose(0, 1, 3, 2)
            .astype(np.float32)
            .reshape(H, W, 4)
        )
        outs.append(o)
    return np.stack(outs, axis=0), res


def kernel(pixel_colors, zbuf, dists, pix_to_face):
    out, _ = _run(pixel_colors, zbuf, dists, pix_to_face, trace=False)
    return out


# revision 33
# speedup vs baseline: 2.2997x; 1.1777x over previous
"""Trainium2 Bass kernel for softmax RGB blend (pytorch3d NoLightShader).

Full inputs (N=8, H=512, W=512, K=8) are sharded batch-wise across 8
NeuronCores (one image per core); the blend is per-pixel, no cross-core
communication.

Host-side input encoding (per core):
    mask folded into the data (pix_to_face never shipped):
        d_eff = where(mask, dists, 1.0)        -> sigmoid(-d/SIGMA) = 0
        z_inv = (ZFAR - zbuf)/(ZFAR - ZNEAR) * mask
    z shipped as uint16 fixed point (z16 = round(65535 * z_inv)): u16 order
    matches float order, so the K-max runs in u16, and ACT's free affine
    (scale/bias) turns u16 straight into exp arguments.
    dists/colors shipped as bf16. Per-tile layout is k-major [K, T] (colors
    [3, K, T]) so every K-reduction is a contiguous pairwise fold tree at
    DVE 2x bf16 mode (tensor_reduce is stuck at 1x). dists ship as one
    up-front stream so ALL sigmoids run in a prepass -- the sigmoid and
    ln/exp ACT table sets otherwise swap twice per tile (~2.7us a load).
    Output is planar bf16 [4, T] per tile (r|g|b|a), host transposes.

Math per pixel:  p_k = sigmoid(-d_k/SIGMA); q_k = 1-p_k
    alpha = 1 - prod_k q_k     (DVE computes mq=p-1; GPSIMD mult fold tree;
                                8 negations cancel)
    zmax  = max_k z_k          (DVE u16 max fold tree)
    w_k   = p_k * exp((z_k - zmax)/GAMMA)  (zd=zmax-z fp16 on GPSIMD, exp ACT)
    delta = exp((EPS - zmax)/GAMMA)
    denom = sum_k w_k + delta              (DVE bf16 add fold tree)
    rgb   = (sum_k w_k c_k + delta)/denom  (bg=1; wc + fold tree on DVE)
    out   = [rgb, alpha]

Engines: SP HWDGE DMAs (d-stream + 1 in + 1 out per tile) | ACT: sigmoid
prepass, exp(zd), delta, ln(denom), rcp=exp(-ln), alpha | DVE: zmax folds,
mq, w, wc, wsum folds, denom, csum folds, t3, rgb | GPSIMD: zd, prod-q
folds. Raw bass, two-pass mark/wait scheduling, double-buffered tiles.
"""

import sys
from contextlib import ExitStack

import numpy as np

if "/opt/trn_rl_repo" not in sys.path:
    sys.path.insert(0, "/opt/trn_rl_repo")

SIGMA = 1e-4
GAMMA = 1e-4
ZNEAR = 1.0
ZFAR = 100.0
EPS = 1e-10

P = 128
K = 8
N_CORES = 8
ROWS = 2048          # H*W / P
T = 256              # pixels per partition per tile
NT = ROWS // T       # 8 tiles
TK = T * K           # 2048
IN_W = TK + TK * 3        # u16 words per tile: z | col
OUT_W = T * 4             # bf16 words per tile (planar r|g|b|a)

S16G = (1.0 / 65535.0) / GAMMA   # u16 step -> 1/GAMMA units


def build_program():
    import concourse.bass as bass
    from concourse import mybir

    dt = mybir.dt
    f32 = dt.float32
    bf16 = dt.bfloat16
    fp16 = dt.float16
    u16 = dt.uint16
    Alu = mybir.AluOpType
    Act = mybir.ActivationFunctionType

    n = NT

    nc = bass.Bass()

    in_d = nc.dram_tensor("inb", [P, n * IN_W], u16, kind="ExternalInput")
    d_d = nc.dram_tensor("din", [P, n * TK], u16, kind="ExternalInput")
    out_d = nc.dram_tensor("out", [P, n * OUT_W], u16, kind="ExternalOutput")

    # const AP for the delta bias (EPS/GAMMA); framework pre-registers 0.0/1.0
    cbias = nc.alloc_sbuf_tensor("c_epsg", [P, 1], f32)
    nc.gpsimd.memset(cbias.ap(), EPS / GAMMA)
    nc.const_aps.aps[(f32, EPS / GAMMA)] = cbias.ap()
    nc.all_engine_barrier()

    with ExitStack() as ctx:
        def sb(name, w, dty=bf16):
            return ctx.enter_context(nc.sbuf_tensor(name, [P, w], dty))

        NB = 3  # input tile buffers
        inb = [sb(f"inb{j}", IN_W, u16) for j in range(NB)]
        d_sb = sb("dall", n * TK, u16)   # d bf16; sigmoid overwrites in place
        ot = [sb(f"ot{j}", OUT_W, u16) for j in range(2)]

        q_b = [sb(f"q{j}", TK) for j in range(2)]
        ex_b = [sb(f"ex{j}", TK) for j in range(2)]
        zd_b = [sb(f"zd{j}", TK, fp16) for j in range(2)]
        zmax = [sb(f"zmax{j}", T, u16) for j in range(2)]
        delta = [sb(f"delta{j}", T) for j in range(2)]
        prodq = [sb(f"prodq{j}", T) for j in range(2)]
        rcp = [sb(f"rcp{j}", T) for j in range(2)]
        t3b = [sb(f"t3{j}", T * 3) for j in range(2)]
        denom = [sb(f"denom{j}", T, f32) for j in range(2)]

        zm4 = sb("zm4", TK // 2, u16)
        zm2 = sb("zm2", TK // 4, u16)
        w_b = sb("w", TK)
        ws4 = sb("ws4", TK // 2)
        ws2 = sb("ws2", TK // 4)
        wsum = sb("wsum", T)
        q4 = sb("q4", TK // 2)
        q2 = sb("q2", TK // 4)
        wc = sb("wc", TK * 3)
        cs4 = sb("cs4", TK * 3 // 2)
        cs2 = sb("cs2", TK * 3 // 4)
        csum = sb("csum", T * 3)
        lnden = sb("lnden", T, f32)

        s_in = [
            ctx.enter_context(nc.semaphore("s_in0")),
            ctx.enter_context(nc.semaphore("s_in1")),
            ctx.enter_context(nc.semaphore("s_in2")),
        ]
        s_out = [
            ctx.enter_context(nc.semaphore("s_out0")),
            ctx.enter_context(nc.semaphore("s_out1")),
        ]
        s_ind = [
            ctx.enter_context(nc.semaphore("s_ind0")),
            ctx.enter_context(nc.semaphore("s_ind1")),
        ]
        s_act = ctx.enter_context(nc.semaphore("s_act"))
        s_dve = ctx.enter_context(nc.semaphore("s_dve"))
        s_gp = ctx.enter_context(nc.semaphore("s_gp"))

        marks = {}

        def mk(engkey, name, t, ctr):
            marks[(engkey, name, t)] = ctr

        # ---- SBUF views -------------------------------------------------
        def z_kt(j):      # [P, K, T] u16
            return inb[j][:, 0:TK].rearrange("p (k t) -> p k t", k=K)

        def col_ckt(j):   # [P, 3, K, T] bf16
            return inb[j][:, TK:IN_W].bitcast(bf16).rearrange(
                "p (c k t) -> p c k t", c=3, k=K
            )

        def d_bf(i):      # [P, TK] bf16, tile i of the d stream
            return d_sb[:, bass.ts(i, TK)].bitcast(bf16)

        def p_t(i):       # [P, TK] bf16, tile i of sigmoid (in-place over d)
            return d_sb[:, bass.ts(i, TK)].bitcast(bf16)

        def ot_rgb(j):    # [P, 3, T] bf16 planar
            return ot[j][:, 0:3 * T].bitcast(bf16).rearrange(
                "p (c t) -> p c t", c=3
            )

        def ot_a(j):      # [P, T] bf16
            return ot[j][:, 3 * T:4 * T].bitcast(bf16)

        # ---- schedules --------------------------------------------------
        def sched_sp(sp):
            if sp is not None:
                # d stream in two halves: sigmoid prepass can start after
                # the first half (each half gets a FULL-value sem wait)
                h = n * TK // 2
                sp.dma_start(out=d_sb[:, 0:h], in_=d_d[:, 0:h]
                             ).then_inc(s_ind[0], 16)
                for i in range(NB):
                    sp.dma_start(
                        out=inb[i][:], in_=in_d[:, bass.ts(i, IN_W)]
                    ).then_inc(s_in[i], 16)
                sp.dma_start(out=d_sb[:, h:2 * h], in_=d_d[:, h:2 * h]
                             ).then_inc(s_ind[1], 16)
            for i in range(NB, n):
                j = i % NB
                if sp is not None:
                    sp.wait_ge(s_dve, marks[("d", "wc", i - NB)])
                    sp.dma_start(
                        out=inb[j][:], in_=in_d[:, bass.ts(i, IN_W)]
                    ).then_inc(s_in[j], 16)
            if sp is not None:
                sp.wait_ge(s_out[0], 16 * ((n + 1) // 2))
                sp.wait_ge(s_out[1], 16 * (n // 2))

        def sched_act(act):
            c = 0
            # sigmoid prepass: one table set, all tiles
            for i in range(n):
                if act is not None:
                    if i == 0:
                        act.wait_ge(s_ind[0], 16)
                    elif i == n // 2:
                        act.wait_ge(s_ind[1], 16)
                    act.activation(
                        p_t(i), d_bf(i), Act.Sigmoid, scale=-1.0 / SIGMA
                    ).then_inc(s_act, 1)
                c += 1; mk("a", "p", i, c)
            for i in range(n + 2):
                t = i - 1
                u = i - 2
                if 0 <= t < n:
                    if act is not None:
                        act.wait_ge(s_dve, marks[("d", "zd", t)])
                        if t >= 2:
                            act.wait_ge(s_dve, marks[("d", "w", t - 2)])
                        act.activation(ex_b[t % 2][:], zd_b[t % 2][:], Act.Exp,
                                       scale=S16G).then_inc(s_act, 1)
                    c += 1; mk("a", "ex", t, c)
                    if act is not None:
                        if t >= 2:
                            act.wait_ge(s_dve, marks[("d", "t3", t - 2)])
                        act.activation(
                            delta[t % 2][:], zmax[t % 2][:], Act.Exp,
                            bias=EPS / GAMMA, scale=-S16G,
                        ).then_inc(s_act, 1)
                    c += 1; mk("a", "delta", t, c)
                if u >= 0:
                    if act is not None:
                        act.wait_ge(s_dve, marks[("d", "denom", u)])
                        act.activation(lnden[:], denom[u % 2][:], Act.Ln
                                       ).then_inc(s_act, 1)
                    c += 1; mk("a", "lnd", u, c)
                    if act is not None:
                        if u >= 2:
                            act.wait_ge(s_dve, marks[("d", "rgb", u - 2)])
                        act.activation(rcp[u % 2][:], lnden[:], Act.Exp,
                                       scale=-1.0).then_inc(s_act, 1)
                    c += 1; mk("a", "rcp", u, c)
                    if act is not None:
                        act.wait_ge(s_dve, marks[("d", "qf3", u)])
                        if u >= 2:
                            act.wait_ge(s_out[u % 2], 16 * (u // 2))
                        act.activation(ot_a(u % 2), prodq[u % 2][:], Act.Copy,
                                       bias=1.0, scale=-1.0).then_inc(s_act, 1)
                    c += 1; mk("a", "alpha", u, c)
                    if act is not None:
                        act.wait_ge(s_dve, marks[("d", "rgb", u)])
                        act.dma_start(
                            out=out_d[:, bass.ts(u, OUT_W)], in_=ot[u % 2][:]
                        ).then_inc(s_out[u % 2], 16)

        def sched_dve(dve):
            c = 0
            for i in range(n + 2):
                t = i - 1
                u = i - 2
                if i < n:
                    j = i % 2
                    jb = i % NB
                    if dve is not None:
                        dve.wait_ge(s_in[jb], 16 * (i // NB + 1))
                        if i >= 2:
                            dve.wait_ge(s_act, marks[("a", "delta", i - 2)])
                        zv = inb[jb][:, 0:TK]
                        dve.tensor_tensor(
                            out=zm4[:], in0=zv[:, 0:TK // 2],
                            in1=zv[:, TK // 2:TK], op=Alu.max,
                        ).then_inc(s_dve, 1)
                    c += 1; mk("d", "zm1", i, c)
                    if dve is not None:
                        dve.tensor_tensor(
                            out=zm2[:], in0=zm4[:, 0:TK // 4],
                            in1=zm4[:, TK // 4:TK // 2], op=Alu.max,
                        ).then_inc(s_dve, 1)
                    c += 1; mk("d", "zm2", i, c)
                    if dve is not None:
                        dve.tensor_tensor(
                            out=zmax[j][:], in0=zm2[:, 0:T],
                            in1=zm2[:, T:2 * T], op=Alu.max,
                        ).then_inc(s_dve, 1)
                    c += 1; mk("d", "zm3", i, c)
                    if dve is not None:
                        if i >= 2:
                            dve.wait_ge(s_act, marks[("a", "ex", i - 2)])
                        dve.tensor_tensor(
                            out=zd_b[j][:].rearrange("p (k t) -> p k t", k=K),
                            in0=z_kt(jb),
                            in1=zmax[j][:].unsqueeze(1).broadcast_to(
                                (P, K, T)),
                            op=Alu.subtract,
                        ).then_inc(s_dve, 1)
                    c += 1; mk("d", "zd", i, c)
                    if dve is not None:
                        dve.wait_ge(s_act, marks[("a", "p", i)])
                        # mq = p - 1 = -q; the 8-way product cancels signs
                        dve.tensor_scalar(
                            out=q_b[j][:], in0=p_t(i), scalar1=1.0,
                            scalar2=None, op0=Alu.subtract,
                        ).then_inc(s_dve, 1)
                    c += 1; mk("d", "q", i, c)
                    if dve is not None:
                        dve.tensor_tensor(
                            out=q4[:], in0=q_b[j][:, 0:TK // 2],
                            in1=q_b[j][:, TK // 2:TK], op=Alu.mult,
                        ).then_inc(s_dve, 1)
                        dve.tensor_tensor(
                            out=q2[:], in0=q4[:, 0:TK // 4],
                            in1=q4[:, TK // 4:TK // 2], op=Alu.mult,
                        ).then_inc(s_dve, 1)
                        if i >= 2:
                            dve.wait_ge(s_act, marks[("a", "alpha", i - 2)])
                        dve.tensor_tensor(
                            out=prodq[j][:], in0=q2[:, 0:T],
                            in1=q2[:, T:2 * T], op=Alu.mult,
                        ).then_inc(s_dve, 1)
                    c += 3; mk("d", "qf3", i, c)
                if 0 <= t < n:
                    jt = t % 2
                    jtb = t % NB
                    if dve is not None:
                        dve.wait_ge(s_act, marks[("a", "ex", t)])
                        dve.tensor_tensor(
                            out=w_b[:], in0=p_t(t), in1=ex_b[jt][:],
                            op=Alu.mult,
                        ).then_inc(s_dve, 1)
                    c += 1; mk("d", "w", t, c)
                    if dve is not None:
                        dve.tensor_tensor(
                            out=wc[:].rearrange("p (c kt) -> p c kt", c=3),
                            in0=inb[jtb][:, TK:IN_W].bitcast(bf16).rearrange(
                                "p (c kt) -> p c kt", c=3),
                            in1=w_b[:].unsqueeze(1).broadcast_to((P, 3, TK)),
                            op=Alu.mult,
                        ).then_inc(s_dve, 1)
                    c += 1; mk("d", "wc", t, c)
                    if dve is not None:
                        dve.tensor_tensor(
                            out=ws4[:], in0=w_b[:, 0:TK // 2],
                            in1=w_b[:, TK // 2:TK], op=Alu.add,
                        ).then_inc(s_dve, 1)
                        dve.tensor_tensor(
                            out=ws2[:], in0=ws4[:, 0:TK // 4],
                            in1=ws4[:, TK // 4:TK // 2], op=Alu.add,
                        ).then_inc(s_dve, 1)
                        dve.tensor_tensor(
                            out=wsum[:], in0=ws2[:, 0:T],
                            in1=ws2[:, T:2 * T], op=Alu.add,
                        ).then_inc(s_dve, 1)
                    c += 3; mk("d", "wsum", t, c)
                    if dve is not None:
                        dve.wait_ge(s_act, marks[("a", "delta", t)])
                        dve.tensor_tensor(
                            out=denom[jt][:], in0=wsum[:], in1=delta[jt][:],
                            op=Alu.add,
                        ).then_inc(s_dve, 1)
                    c += 1; mk("d", "denom", t, c)
                    if dve is not None:
                        wcv = wc[:].rearrange("p (c k t) -> p c k t", c=3, k=K)
                        dve.tensor_tensor(
                            out=cs4[:].rearrange("p (c k t) -> p c k t",
                                                 c=3, k=K // 2),
                            in0=wcv[:, :, 0:K // 2, :],
                            in1=wcv[:, :, K // 2:K, :], op=Alu.add,
                        ).then_inc(s_dve, 1)
                        cs4v = cs4[:].rearrange("p (c k t) -> p c k t",
                                                c=3, k=K // 2)
                        dve.tensor_tensor(
                            out=cs2[:].rearrange("p (c k t) -> p c k t",
                                                 c=3, k=K // 4),
                            in0=cs4v[:, :, 0:K // 4, :],
                            in1=cs4v[:, :, K // 4:K // 2, :], op=Alu.add,
                        ).then_inc(s_dve, 1)
                        cs2v = cs2[:].rearrange("p (c k t) -> p c k t",
                                                c=3, k=K // 4)
                        dve.tensor_tensor(
                            out=csum[:].rearrange("p (c t) -> p c t", c=3),
                            in0=cs2v[:, :, 0, :],
                            in1=cs2v[:, :, 1, :], op=Alu.add,
                        ).then_inc(s_dve, 1)
                    c += 3; mk("d", "csum", t, c)
                    if dve is not None:
                        dve.tensor_tensor(
                            out=t3b[jt][:].rearrange("p (c t) -> p c t", c=3),
                            in0=csum[:].rearrange("p (c t) -> p c t", c=3),
                            in1=delta[jt][:].unsqueeze(1).broadcast_to(
                                (P, 3, T)),
                            op=Alu.add,
                        ).then_inc(s_dve, 1)
                    c += 1; mk("d", "t3", t, c)
                if 0 <= u:
                    ju = u % 2
                    if dve is not None:
                        dve.wait_ge(s_act, marks[("a", "rcp", u)])
                        if u >= 2:
                            dve.wait_ge(s_out[ju], 16 * (u // 2))
                        dve.tensor_tensor(
                            out=ot_rgb(ju),
                            in0=t3b[ju][:].rearrange("p (c t) -> p c t", c=3),
                            in1=rcp[ju][:].unsqueeze(1).broadcast_to(
                                (P, 3, T)),
                            op=Alu.mult,
                        ).then_inc(s_dve, 1)
                    c += 1; mk("d", "rgb", u, c)

        # pass 1: record marks
        sched_sp(None)
        sched_act(None)
        sched_dve(None)

        blk = ctx.enter_context(nc.Block())

        @blk.sync
        def _(sp):
            sched_sp(sp)

        @blk.scalar
        def _(act):
            sched_act(act)

        @blk.vector
        def _(dve):
            sched_dve(dve)

    return nc


_CACHE = {}


def _get_program():
    if "nc" not in _CACHE:
        _CACHE["nc"] = build_program()
    return _CACHE["nc"]


def _pack_core(zb, ds, pf, pc, bf16_t):
    """Per-core input: [P, NT*IN_W] u16 blob (z|col) and [P, NT*TK] d."""
    mask = pf >= 0
    z_inv = (ZFAR - zb) * (np.float32(1.0) / (ZFAR - ZNEAR))
    z_inv = np.where(mask, z_inv, np.float32(0.0))
    z16 = np.clip(np.rint(z_inv * np.float32(65535.0)), 0, 65535).astype(
        np.uint16
    )
    d_eff = np.where(mask, ds, np.float32(1.0)).astype(bf16_t).view(np.uint16)

    # pixel p-major: (H*W, K[,3]) -> [P, NT, ...] k-major tiles
    z16 = (
        z16.reshape(P, NT, T, K).transpose(0, 1, 3, 2).reshape(P, NT, TK)
    )
    d16 = (
        d_eff.reshape(P, NT, T, K).transpose(0, 1, 3, 2).reshape(P, NT * TK)
    )
    c16 = (
        pc.astype(bf16_t)
        .view(np.uint16)
        .reshape(P, NT, T, K, 3)
        .transpose(0, 1, 4, 3, 2)
        .reshape(P, NT, TK * 3)
    )
    blob = np.ascontiguousarray(
        np.concatenate([z16, c16], axis=2)
    ).reshape(P, NT * IN_W)
    return blob, np.ascontiguousarray(d16)


def _run(pixel_colors, zbuf, dists, pix_to_face, trace=False):
    import ml_dtypes
    from concourse.bass_utils import run_bass_kernel_spmd

    bf16_t = ml_dtypes.bfloat16

    N, H, W, Kk = zbuf.shape
    assert (N, H, W, Kk) == (N_CORES, 512, 512, K), (N, H, W, Kk)

    nc = _get_program()

    pc = np.asarray(pixel_colors, dtype=np.float32)
    zb = np.asarray(zbuf, dtype=np.float32)
    ds = np.asarray(dists, dtype=np.float32)
    pf = np.asarray(pix_to_face)

    in_maps = []
    for i in range(N_CORES):
        blob, din = _pack_core(
            zb[i].reshape(-1, K),
            ds[i].reshape(-1, K),
            pf[i].reshape(-1, K),
            pc[i].reshape(-1, K, 3),
            bf16_t,
        )
        in_maps.append({"inb": blob, "din": din})

    res = run_bass_kernel_spmd(
        nc, in_maps, core_ids=list(range(N_CORES)), trace=trace
    )
    outs = []
    for i in range(N_CORES):
        o = res.results[i]["out"]  # [P, NT*OUT_W] u16
        o = (
            np.ascontiguousarray(o)
            .view(bf16_t)
            .reshape(P, NT, 4, T)
            .transpose(0, 1, 3, 2)
            .astype(np.float32)
            .reshape(H, W, 4)
        )
        outs.append(o)
    return np.stack(outs, axis=0), res


def kernel(pixel_colors, zbuf, dists, pix_to_face):
    out, _ = _run(pixel_colors, zbuf, dists, pix_to_face, trace=False)
    return out


# revision 39
# speedup vs baseline: 2.5508x; 1.1092x over previous
"""Trainium2 Bass kernel for softmax RGB blend (pytorch3d NoLightShader).

Full inputs (N=8, H=512, W=512, K=8) are sharded batch-wise across 8
NeuronCores (one image per core); the blend is per-pixel, no cross-core
communication.

Host-side input encoding (per core):
    mask folded into the data (pix_to_face never shipped):
        d_eff = where(mask, dists, 1.0)        -> sigmoid(-d/SIGMA) = 0
        z_inv = (ZFAR - zbuf)/(ZFAR - ZNEAR) * mask
    z shipped as uint16 fixed point (z16 = round(65535 * z_inv)): u16 order
    matches float order, so the K-max runs in u16, and ACT's free affine
    (scale/bias) turns u16 straight into exp arguments.
    dists/colors shipped as bf16. Per-tile layout is k-major [K, T] (colors
    [3, K, T]) so every K-reduction is a contiguous pairwise fold tree at
    DVE 2x bf16 mode (tensor_reduce is stuck at 1x). dists ship as one
    up-front stream so ALL sigmoids run in a prepass -- the sigmoid and
    ln/exp ACT table sets otherwise swap twice per tile (~2.7us a load).
    Output is planar bf16 [4, T] per tile (r|g|b|a), host transposes.

Math per pixel:  p_k = sigmoid(-d_k/SIGMA); q_k = 1-p_k
    alpha = 1 - prod_k q_k     (DVE computes mq=p-1; GPSIMD mult fold tree;
                                8 negations cancel)
    zmax  = max_k z_k          (DVE u16 max fold tree)
    w_k   = p_k * exp((z_k - zmax)/GAMMA)  (zd=zmax-z fp16 on GPSIMD, exp ACT)
    delta = exp((EPS - zmax)/GAMMA)
    denom = sum_k w_k + delta              (DVE bf16 add fold tree)
    rgb   = (sum_k w_k c_k + delta)/denom  (bg=1; wc + fold tree on DVE)
    out   = [rgb, alpha]

Engines: SP HWDGE DMAs (d-stream + 1 in + 1 out per tile) | ACT: sigmoid
prepass, exp(zd), delta, ln(denom), rcp=exp(-ln), alpha | DVE: zmax folds,
mq, w, wc, wsum folds, denom, csum folds, t3, rgb | GPSIMD: zd, prod-q
folds. Raw bass, two-pass mark/wait scheduling, double-buffered tiles.
"""

import sys
from contextlib import ExitStack

import numpy as np

if "/opt/trn_rl_repo" not in sys.path:
    sys.path.insert(0, "/opt/trn_rl_repo")

SIGMA = 1e-4
GAMMA = 1e-4
ZNEAR = 1.0
ZFAR = 100.0
EPS = 1e-10

P = 128
K = 8
N_CORES = 8
ROWS = 2048          # H*W / P
T = 256              # pixels per partition per tile
NT = ROWS // T       # 8 tiles
TK = T * K           # 2048
IN_W = TK + TK * 3        # u16 words per tile: z | col
OUT_W = T * 4             # bf16 words per tile (planar r|g|b|a)

S16G = (1.0 / 65535.0) / GAMMA   # u16 step -> 1/GAMMA units


def build_program():
    import concourse.bass as bass
    from concourse import mybir

    dt = mybir.dt
    f32 = dt.float32
    bf16 = dt.bfloat16
    fp16 = dt.float16
    u16 = dt.uint16
    Alu = mybir.AluOpType
    Act = mybir.ActivationFunctionType

    n = NT

    nc = bass.Bass()

    in_d = nc.dram_tensor("inb", [P, n * IN_W], u16, kind="ExternalInput")
    d_d = nc.dram_tensor("din", [P, n * TK], u16, kind="ExternalInput")
    out_d = nc.dram_tensor("out", [P, n * OUT_W], u16, kind="ExternalOutput")

    # const AP for the delta bias (EPS/GAMMA); framework pre-registers 0.0/1.0.
    # Written by the first DVE op; every ACT reader (delta) transitively waits
    # on later DVE marks, so no barrier is needed.
    cbias = nc.alloc_sbuf_tensor("c_epsg", [P, 1], f32)
    nc.const_aps.aps[(f32, EPS / GAMMA)] = cbias.ap()

    with ExitStack() as ctx:
        def sb(name, w, dty=bf16):
            return ctx.enter_context(nc.sbuf_tensor(name, [P, w], dty))

        NB = 3  # input tile buffers
        inb = [sb(f"inb{j}", IN_W, u16) for j in range(NB)]
        d_sb = sb("dall", n * TK, u16)   # d bf16; sigmoid overwrites in place
        ot = [sb(f"ot{j}", OUT_W, u16) for j in range(2)]

        q_b = [sb(f"q{j}", TK) for j in range(2)]
        ex_b = [sb(f"ex{j}", TK) for j in range(2)]
        zd_b = [sb(f"zd{j}", TK, fp16) for j in range(2)]
        zmax = [sb(f"zmax{j}", T, u16) for j in range(2)]
        delta = [sb(f"delta{j}", T) for j in range(2)]
        qsum = [sb(f"qsum{j}", T, f32) for j in range(2)]
        pqt = sb("pqt", T, f32)
        rcp = [sb(f"rcp{j}", T) for j in range(2)]
        t3b = [sb(f"t3{j}", T * 3) for j in range(2)]
        denom = [sb(f"denom{j}", T, f32) for j in range(2)]

        zm4 = sb("zm4", TK // 2, u16)
        zm2 = sb("zm2", TK // 4, u16)
        w_b = sb("w", TK)
        ws4 = sb("ws4", TK // 2)
        ws2 = sb("ws2", TK // 4)
        wsum = sb("wsum", T)
        q4 = sb("q4", TK // 2)
        q2 = sb("q2", TK // 4)
        wc = sb("wc", TK * 3)
        cs4 = sb("cs4", TK * 3 // 2)
        cs2 = sb("cs2", TK * 3 // 4)
        csum = sb("csum", T * 3)
        lnden = sb("lnden", T, f32)

        s_in = [
            ctx.enter_context(nc.semaphore("s_in0")),
            ctx.enter_context(nc.semaphore("s_in1")),
            ctx.enter_context(nc.semaphore("s_in2")),
        ]
        s_out = [
            ctx.enter_context(nc.semaphore("s_out0")),
            ctx.enter_context(nc.semaphore("s_out1")),
        ]
        s_ind = [
            ctx.enter_context(nc.semaphore("s_ind0")),
            ctx.enter_context(nc.semaphore("s_ind1")),
        ]
        s_act = ctx.enter_context(nc.semaphore("s_act"))
        s_dve = ctx.enter_context(nc.semaphore("s_dve"))
        s_gp = ctx.enter_context(nc.semaphore("s_gp"))

        marks = {}

        def mk(engkey, name, t, ctr):
            marks[(engkey, name, t)] = ctr

        # ---- SBUF views -------------------------------------------------
        def z_kt(j):      # [P, K, T] u16
            return inb[j][:, 0:TK].rearrange("p (k t) -> p k t", k=K)

        def col_ckt(j):   # [P, 3, K, T] bf16
            return inb[j][:, TK:IN_W].bitcast(bf16).rearrange(
                "p (c k t) -> p c k t", c=3, k=K
            )

        def d_bf(i):      # [P, TK] bf16, tile i of the d stream
            return d_sb[:, bass.ts(i, TK)].bitcast(bf16)

        def p_t(i):       # [P, TK] bf16, tile i of sigmoid (in-place over d)
            return d_sb[:, bass.ts(i, TK)].bitcast(bf16)

        def ot_rgb(j):    # [P, 3, T] bf16 planar
            return ot[j][:, 0:3 * T].bitcast(bf16).rearrange(
                "p (c t) -> p c t", c=3
            )

        def ot_a(j):      # [P, T] bf16
            return ot[j][:, 3 * T:4 * T].bitcast(bf16)

        # ---- schedules --------------------------------------------------
        def sched_sp(sp):
            if sp is not None:
                # d stream in two halves: sigmoid prepass can start after
                # the first half (each half gets a FULL-value sem wait)
                h = n * TK // 2
                sp.dma_start(out=d_sb[:, 0:h], in_=d_d[:, 0:h]
                             ).then_inc(s_ind[0], 16)
                for i in range(NB):
                    sp.dma_start(
                        out=inb[i][:], in_=in_d[:, bass.ts(i, IN_W)]
                    ).then_inc(s_in[i], 16)
                sp.dma_start(out=d_sb[:, h:2 * h], in_=d_d[:, h:2 * h]
                             ).then_inc(s_ind[1], 16)
            for i in range(NB, n):
                j = i % NB
                if sp is not None:
                    sp.wait_ge(s_dve, marks[("d", "wc", i - NB)])
                    sp.dma_start(
                        out=inb[j][:], in_=in_d[:, bass.ts(i, IN_W)]
                    ).then_inc(s_in[j], 16)
            if sp is not None:
                sp.wait_ge(s_out[0], 16 * ((n + 1) // 2))
                sp.wait_ge(s_out[1], 16 * (n // 2))

        def emit_ex(act, t):
            act.activation(ex_b[t % 2][:], zd_b[t % 2][:], Act.Exp,
                           scale=S16G).then_inc(s_act, 1)

        def emit_delta(act, t):
            act.activation(
                delta[t % 2][:], zmax[t % 2][:], Act.Exp,
                bias=EPS / GAMMA, scale=-S16G,
            ).then_inc(s_act, 1)

        def emit_lnq(act, t):
            act.activation(q_b[t % 2][:], p_t(t), Act.Ln,
                           bias=1.0, scale=-1.0).then_inc(s_act, 1)

        def sched_act(act):
            c = 0
            # sigmoid prepass (one table set); tile 0's exp/ln ops are
            # interleaved after sig(2) so DVE's w(0) isn't blocked on the
            # whole prepass
            for i in range(n):
                if act is not None:
                    if i == 0:
                        act.wait_ge(s_ind[0], 16)
                    elif i == n // 2:
                        act.wait_ge(s_ind[1], 16)
                    act.activation(
                        p_t(i), d_bf(i), Act.Sigmoid, scale=-1.0 / SIGMA
                    ).then_inc(s_act, 1)
                c += 1; mk("a", "p", i, c)
                if i == 2:
                    if act is not None:
                        act.wait_ge(s_dve, marks[("d", "zd", 0)])
                        emit_ex(act, 0)
                    c += 1; mk("a", "ex", 0, c)
                    if act is not None:
                        emit_delta(act, 0)
                    c += 1; mk("a", "delta", 0, c)
                    if act is not None:
                        emit_lnq(act, 0)
                    c += 1; mk("a", "lnq", 0, c)
            for i in range(n + 2):
                t = i - 1
                u = i - 2
                if 1 <= t < n:
                    if act is not None:
                        act.wait_ge(s_dve, marks[("d", "zd", t)])
                        if t >= 2:
                            act.wait_ge(s_dve, marks[("d", "w", t - 2)])
                        emit_ex(act, t)
                    c += 1; mk("a", "ex", t, c)
                    if act is not None:
                        if t >= 2:
                            act.wait_ge(s_dve, marks[("d", "t3", t - 2)])
                        emit_delta(act, t)
                    c += 1; mk("a", "delta", t, c)
                    if act is not None:
                        emit_lnq(act, t)
                    c += 1; mk("a", "lnq", t, c)
                if u >= 0:
                    if act is not None:
                        act.wait_ge(s_dve, marks[("d", "denom", u)])
                        act.activation(lnden[:], denom[u % 2][:], Act.Ln
                                       ).then_inc(s_act, 1)
                    c += 1; mk("a", "lnd", u, c)
                    if act is not None:
                        if u >= 2:
                            act.wait_ge(s_dve, marks[("d", "rgb", u - 2)])
                        act.activation(rcp[u % 2][:], lnden[:], Act.Exp,
                                       scale=-1.0).then_inc(s_act, 1)
                    c += 1; mk("a", "rcp", u, c)
                    if act is not None:
                        act.wait_ge(s_dve, marks[("d", "qsum", u)])
                        act.activation(pqt[:], qsum[u % 2][:], Act.Exp
                                       ).then_inc(s_act, 1)
                    c += 1; mk("a", "pq", u, c)
                    if act is not None:
                        if u >= 2:
                            act.wait_ge(s_out[u % 2], 16 * (u // 2))
                        act.activation(ot_a(u % 2), pqt[:], Act.Copy,
                                       bias=1.0, scale=-1.0).then_inc(s_act, 1)
                    c += 1; mk("a", "alpha", u, c)
                    if act is not None:
                        act.wait_ge(s_dve, marks[("d", "rgb", u)])
                        act.dma_start(
                            out=out_d[:, bass.ts(u, OUT_W)], in_=ot[u % 2][:]
                        ).then_inc(s_out[u % 2], 16)

        def sched_dve(dve):
            c = 0
            if dve is not None:
                dve.memset(cbias.ap(), EPS / GAMMA)
            for i in range(n + 2):
                t = i - 1
                u = i - 2
                if i < n:
                    j = i % 2
                    jb = i % NB
                    if dve is not None:
                        dve.wait_ge(s_in[jb], 16 * (i // NB + 1))
                        if i >= 2:
                            dve.wait_ge(s_act, marks[("a", "delta", i - 2)])
                        zv = inb[jb][:, 0:TK]
                        dve.tensor_tensor(
                            out=zm4[:], in0=zv[:, 0:TK // 2],
                            in1=zv[:, TK // 2:TK], op=Alu.max,
                        ).then_inc(s_dve, 1)
                    c += 1; mk("d", "zm1", i, c)
                    if dve is not None:
                        dve.tensor_tensor(
                            out=zm2[:], in0=zm4[:, 0:TK // 4],
                            in1=zm4[:, TK // 4:TK // 2], op=Alu.max,
                        ).then_inc(s_dve, 1)
                    c += 1; mk("d", "zm2", i, c)
                    if dve is not None:
                        dve.tensor_tensor(
                            out=zmax[j][:], in0=zm2[:, 0:T],
                            in1=zm2[:, T:2 * T], op=Alu.max,
                        ).then_inc(s_dve, 1)
                    c += 1; mk("d", "zm3", i, c)
                    if dve is not None:
                        if i >= 2:
                            dve.wait_ge(s_act, marks[("a", "ex", i - 2)])
                        dve.tensor_tensor(
                            out=zd_b[j][:].rearrange("p (k t) -> p k t", k=K),
                            in0=z_kt(jb),
                            in1=zmax[j][:].unsqueeze(1).broadcast_to(
                                (P, K, T)),
                            op=Alu.subtract,
                        ).then_inc(s_dve, 1)
                    c += 1; mk("d", "zd", i, c)
                if 0 <= t < n:
                    jt = t % 2
                    jtb = t % NB
                    if dve is not None:
                        dve.wait_ge(s_act, marks[("a", "ex", t)])
                        dve.tensor_tensor(
                            out=w_b[:], in0=p_t(t), in1=ex_b[jt][:],
                            op=Alu.mult,
                        ).then_inc(s_dve, 1)
                    c += 1; mk("d", "w", t, c)
                    if dve is not None:
                        dve.tensor_tensor(
                            out=wc[:].rearrange("p (c kt) -> p c kt", c=3),
                            in0=inb[jtb][:, TK:IN_W].bitcast(bf16).rearrange(
                                "p (c kt) -> p c kt", c=3),
                            in1=w_b[:].unsqueeze(1).broadcast_to((P, 3, TK)),
                            op=Alu.mult,
                        ).then_inc(s_dve, 1)
                    c += 1; mk("d", "wc", t, c)
                    if dve is not None:
                        dve.tensor_tensor(
                            out=ws4[:], in0=w_b[:, 0:TK // 2],
                            in1=w_b[:, TK // 2:TK], op=Alu.add,
                        ).then_inc(s_dve, 1)
                        dve.tensor_tensor(
                            out=ws2[:], in0=ws4[:, 0:TK // 4],
                            in1=ws4[:, TK // 4:TK // 2], op=Alu.add,
                        ).then_inc(s_dve, 1)
                        dve.tensor_tensor(
                            out=wsum[:], in0=ws2[:, 0:T],
                            in1=ws2[:, T:2 * T], op=Alu.add,
                        ).then_inc(s_dve, 1)
                    c += 3; mk("d", "wsum", t, c)
                    if dve is not None:
                        dve.wait_ge(s_act, marks[("a", "delta", t)])
                        dve.tensor_tensor(
                            out=denom[jt][:], in0=wsum[:], in1=delta[jt][:],
                            op=Alu.add,
                        ).then_inc(s_dve, 1)
                    c += 1; mk("d", "denom", t, c)
                    if dve is not None:
                        dve.wait_ge(s_act, marks[("a", "lnq", t)])
                        dve.tensor_tensor(
                            out=q4[:], in0=q_b[jt][:, 0:TK // 2],
                            in1=q_b[jt][:, TK // 2:TK], op=Alu.add,
                        ).then_inc(s_dve, 1)
                        dve.tensor_tensor(
                            out=q2[:], in0=q4[:, 0:TK // 4],
                            in1=q4[:, TK // 4:TK // 2], op=Alu.add,
                        ).then_inc(s_dve, 1)
                        if t >= 2:
                            dve.wait_ge(s_act, marks[("a", "pq", t - 2)])
                        dve.tensor_tensor(
                            out=qsum[jt][:], in0=q2[:, 0:T],
                            in1=q2[:, T:2 * T], op=Alu.add,
                        ).then_inc(s_dve, 1)
                    c += 3; mk("d", "qsum", t, c)
                    if dve is not None:
                        wcv = wc[:].rearrange("p (c k t) -> p c k t", c=3, k=K)
                        dve.tensor_tensor(
                            out=cs4[:].rearrange("p (c k t) -> p c k t",
                                                 c=3, k=K // 2),
                            in0=wcv[:, :, 0:K // 2, :],
                            in1=wcv[:, :, K // 2:K, :], op=Alu.add,
                        ).then_inc(s_dve, 1)
                        cs4v = cs4[:].rearrange("p (c k t) -> p c k t",
                                                c=3, k=K // 2)
                        dve.tensor_tensor(
                            out=cs2[:].rearrange("p (c k t) -> p c k t",
                                                 c=3, k=K // 4),
                            in0=cs4v[:, :, 0:K // 4, :],
                            in1=cs4v[:, :, K // 4:K // 2, :], op=Alu.add,
                        ).then_inc(s_dve, 1)
                        cs2v = cs2[:].rearrange("p (c k t) -> p c k t",
                                                c=3, k=K // 4)
                        dve.tensor_tensor(
                            out=csum[:].rearrange("p (c t) -> p c t", c=3),
                            in0=cs2v[:, :, 0, :],
                            in1=cs2v[:, :, 1, :], op=Alu.add,
                        ).then_inc(s_dve, 1)
                    c += 3; mk("d", "csum", t, c)
                    if dve is not None:
                        dve.tensor_tensor(
                            out=t3b[jt][:].rearrange("p (c t) -> p c t", c=3),
                            in0=csum[:].rearrange("p (c t) -> p c t", c=3),
                            in1=delta[jt][:].unsqueeze(1).broadcast_to(
                                (P, 3, T)),
                            op=Alu.add,
                        ).then_inc(s_dve, 1)
                    c += 1; mk("d", "t3", t, c)
                if 0 <= u:
                    ju = u % 2
                    if dve is not None:
                        dve.wait_ge(s_act, marks[("a", "rcp", u)])
                        if u >= 2:
                            dve.wait_ge(s_out[ju], 16 * (u // 2))
                        dve.tensor_tensor(
                            out=ot_rgb(ju),
                            in0=t3b[ju][:].rearrange("p (c t) -> p c t", c=3),
                            in1=rcp[ju][:].unsqueeze(1).broadcast_to(
                                (P, 3, T)),
                            op=Alu.mult,
                        ).then_inc(s_dve, 1)
                    c += 1; mk("d", "rgb", u, c)

        # pass 1: record marks
        sched_sp(None)
        sched_act(None)
        sched_dve(None)

        blk = ctx.enter_context(nc.Block())

        @blk.sync
        def _(sp):
            sched_sp(sp)

        @blk.scalar
        def _(act):
            sched_act(act)

        @blk.vector
        def _(dve):
            sched_dve(dve)

    return nc


_CACHE = {}


def _get_program():
    if "nc" not in _CACHE:
        _CACHE["nc"] = build_program()
    return _CACHE["nc"]


def _pack_core(zb, ds, pf, pc, bf16_t):
    """Per-core input: [P, NT*IN_W] u16 blob (z|col) and [P, NT*TK] d."""
    mask = pf >= 0
    z_inv = (ZFAR - zb) * (np.float32(1.0) / (ZFAR - ZNEAR))
    z_inv = np.where(mask, z_inv, np.float32(0.0))
    z16 = np.clip(np.rint(z_inv * np.float32(65535.0)), 0, 65535).astype(
        np.uint16
    )
    d_eff = np.where(mask, ds, np.float32(1.0)).astype(bf16_t).view(np.uint16)

    # pixel p-major: (H*W, K[,3]) -> [P, NT, ...] k-major tiles
    z16 = (
        z16.reshape(P, NT, T, K).transpose(0, 1, 3, 2).reshape(P, NT, TK)
    )
    d16 = (
        d_eff.reshape(P, NT, T, K).transpose(0, 1, 3, 2).reshape(P, NT * TK)
    )
    c16 = (
        pc.astype(bf16_t)
        .view(np.uint16)
        .reshape(P, NT, T, K, 3)
        .transpose(0, 1, 4, 3, 2)
        .reshape(P, NT, TK * 3)
    )
    blob = np.ascontiguousarray(
        np.concatenate([z16, c16], axis=2)
    ).reshape(P, NT * IN_W)
    return blob, np.ascontiguousarray(d16)


def _run(pixel_colors, zbuf, dists, pix_to_face, trace=False):
    import ml_dtypes
    from concourse.bass_utils import run_bass_kernel_spmd

    bf16_t = ml_dtypes.bfloat16

    N, H, W, Kk = zbuf.shape
    assert (N, H, W, Kk) == (N_CORES, 512, 512, K), (N, H, W, Kk)

    nc = _get_program()

    pc = np.asarray(pixel_colors, dtype=np.float32)
    zb = np.asarray(zbuf, dtype=np.float32)
    ds = np.asarray(dists, dtype=np.float32)
    pf = np.asarray(pix_to_face)

    in_maps = []
    for i in range(N_CORES):
        blob, din = _pack_core(
            zb[i].reshape(-1, K),
            ds[i].reshape(-1, K),
            pf[i].reshape(-1, K),
            pc[i].reshape(-1, K, 3),
            bf16_t,
        )
        in_maps.append({"inb": blob, "din": din})

    res = run_bass_kernel_spmd(
        nc, in_maps, core_ids=list(range(N_CORES)), trace=trace
    )
    outs = []
    for i in range(N_CORES):
        o = res.results[i]["out"]  # [P, NT*OUT_W] u16
        o = (
            np.ascontiguousarray(o)
            .view(bf16_t)
            .reshape(P, NT, 4, T)
            .transpose(0, 1, 3, 2)
            .astype(np.float32)
            .reshape(H, W, 4)
        )
        outs.append(o)
    return np.stack(outs, axis=0), res


def kernel(pixel_colors, zbuf, dists, pix_to_face):
    out, _ = _run(pixel_colors, zbuf, dists, pix_to_face, trace=False)
    return out
